# revision 1
# baseline (speedup 1.0000x reference)
"""GSA video block kernel for 8 TRN2 NeuronCores.

Sharding: head-parallel attention (2 heads/core) -> one AllToAll that
redistributes the RMS-normed head outputs from head-sharded to
token-sharded -> token-parallel tail (out-proj + LN2 + MLP with full
weights, 256 tokens/core).

The sequential T=512 gated-slot-attention scan is replaced by an exact
chunk-parallel formulation (C=128): intra-chunk terms via causal-masked
matmuls with per-slot decay factors, inter-chunk via carried states
K[DK,M] / V[M,DV].
"""

import os
import sys

import numpy as np
import ml_dtypes

if "/opt/trn_rl_repo" not in sys.path:
    sys.path.insert(0, "/opt/trn_rl_repo")

import concourse.bass as bass  # noqa: E402
import concourse.mybir as mybir  # noqa: E402
import concourse.tile as tile  # noqa: E402
from concourse import bacc  # noqa: E402
from concourse.bass_utils import run_bass_kernel_spmd  # noqa: E402

BF16 = mybir.dt.bfloat16
F32 = mybir.dt.float32
AF = mybir.ActivationFunctionType
ALU = mybir.AluOpType
AX = mybir.AxisListType

B, T, D = 4, 512, 1024
H, DK, DV, M = 16, 64, 64, 64
MLP = 4096
EPS = 1e-6

N_CORES = 8
C = 128                    # scan chunk length
NCH = T // C               # chunks per batch = 4
TOK = B * T                # 2048 flat tokens
TT = TOK // 128            # 16 token tiles
DT = D // 128              # 8 d tiles
MT = MLP // 128            # 32 mlp tiles
TAIL = TOK // N_CORES      # 256 tokens per core in the tail
LN8 = float(np.log(0.125))
RG = [list(range(N_CORES))]

_cache = {}


def _emit(nc, tc, io):
    x_bf, x_res = io["x_bf"], io["x_res"]
    wq, wk, wv, wf = io["wq"], io["wk"], io["wv"], io["wf"]
    bqp, bkp = io["bqp"], io["bkp"]
    bvp, bfp, b1row = io["bvp"], io["bfp"], io["b1row"]
    wo, w1, b1v, w2, b2v = io["wo"], io["w1"], io["b1v"], io["w2"], io["b2v"]
    ltriT, onescol, cmask = io["ltriT"], io["onescol"], io["cmask"]
    ident, bd128, ones_row = io["ident"], io["bd128"], io["ones_row"]
    y_out, dump = io["y_out"], io["dump"]
    P = 128

    const = tc.alloc_tile_pool(name="const", bufs=1)
    persist = tc.alloc_tile_pool(name="persist", bufs=1)
    dram = tc.alloc_tile_pool(name="dram", bufs=1, space="DRAM")

    # ---- warmup collective (prepay ncfw handshake) -----------------------
    wa_in = dram.tile([8, 128], BF16, name="wa_in")
    wa_out = dram.tile([8, 128], BF16, name="wa_out")
    nc.gpsimd.collective_compute("AllReduce", ALU.add, replica_groups=RG,
                                 ins=[wa_in.opt()], outs=[wa_out.opt()])

    # ---- constants into SBUF --------------------------------------------
    def cload(ap, shape, dt, name):
        t = const.tile(shape, dt, name=name)
        nc.sync.dma_start(t[:], ap)
        return t

    ltriT_sb = cload(ltriT.ap(), [128, 128], F32, "ltriT")
    onescol_sb = cload(onescol.ap(), [128, 1], F32, "onescol")
    cmask_sb = cload(cmask.ap(), [128, 128], BF16, "cmask")
    ident_sb = cload(ident.ap(), [128, 128], BF16, "ident")
    bd128_sb = cload(bd128.ap(), [128, 128], BF16, "bd128")
    ones_row_sb = cload(ones_row.ap(), [1, 128], BF16, "ones_row")
    bqp_sb = cload(bqp.ap(), [128, 1], F32, "bqp")
    bkp_sb = cload(bkp.ap(), [128, 1], F32, "bkp")
    bvp_sb = cload(bvp.ap(), [128, 1], F32, "bvp")
    bfp_sb = cload(bfp.ap(), [128, 1], F32, "bfp")
    b1row_sb = cload(b1row.ap(), [1, MLP], BF16, "b1row")
    b1_sb = cload(b1v.ap(), [128, MT], F32, "b1")
    eps_sb = const.tile([128, 1], F32)
    nc.vector.memset(eps_sb[:], EPS)
    ln8_sb = const.tile([128, 1], F32)
    nc.vector.memset(ln8_sb[:], LN8)

    wq_sb = const.tile([128, DT, 128], BF16)
    nc.sync.dma_start(wq_sb[:], wq.ap().rearrange("(dt p) j -> p dt j", p=P))
    wk_sb = const.tile([128, DT, 128], BF16)
    nc.sync.dma_start(wk_sb[:], wk.ap().rearrange("(dt p) j -> p dt j", p=P))
    wv_sb = const.tile([128, DT, 128], BF16)
    nc.sync.dma_start(wv_sb[:], wv.ap().rearrange("(dt p) j -> p dt j", p=P))
    wf_sb = const.tile([128, DT, 128], BF16)
    nc.sync.dma_start(wf_sb[:], wf.ap().rearrange("(dt p) j -> p dt j", p=P))
    wo_sb = const.tile([128, DT, D], BF16)
    nc.sync.dma_start(wo_sb[:], wo.ap().rearrange("(dt p) j -> p dt j", p=P))

    # ---- persistent activation tensors ----------------------------------
    qT = persist.tile([128, TOK], BF16, name="qT")       # [2h*64 dk, t]
    kT = persist.tile([128, TOK], BF16, name="kT")
    k_tm = persist.tile([128, TT, 128], BF16, name="k_tm")   # [t, 2h*64]
    v_tm = persist.tile([128, TT, 128], BF16, name="v_tm")
    vT = persist.tile([128, TOK], BF16, name="vT")
    fT = persist.tile([128, TOK], BF16, name="fT")
    f_tm = persist.tile([128, TT, 128], BF16, name="f_tm")
    sp = persist.tile([128, TT, 128], F32, name="sp")        # softplus(-f)
    s_tm = persist.tile([128, TT, 128], BF16, name="s_tm")   # 1-exp(g)
    onT = persist.tile([128, TOK], BF16, name="onT")         # normed oT

    h_dram = [dram.tile([TOK // 2, D], BF16, name=f"h_dram{g}")
              for g in range(2)]
    k_dram = dram.tile([128, TOK], BF16, name="k_dram")
    v_dram = dram.tile([128, TOK], BF16, name="v_dram")
    f_dram = dram.tile([128, TOK], BF16, name="f_dram")
    z_dram = dram.tile([TAIL, MLP], BF16, name="z_dram")
    a2a_in = dram.tile([128 * N_CORES, TAIL], BF16, name="a2a_in")
    a2a_out = dram.tile([128 * N_CORES, TAIL], BF16, name="a2a_out")
    h2d = dram.tile([TAIL, D], BF16, name="h2d")

    # =====================================================================
    # P1: LN1 stats (batched sqrt) + normalize, store h to DRAM
    # =====================================================================
    p0 = tc.alloc_tile_pool(name="p0", bufs=1)
    x_sb = p0.tile([128, TT, D], BF16, name="x_sb")
    stats = p0.tile([128, 2, TT], F32, name="stats")  # [.,0,:]=r [.,1,:]=nrmu
    with tc.tile_pool(name="p1", bufs=3) as p1, \
         tc.tile_pool(name="p1s", bufs=2) as p1s:
        musb = p1.tile([128, TT], F32, name="musb")
        sssb = p1.tile([128, TT], F32, name="sssb")
        for tt in range(TT):
            nc.sync.dma_start(
                x_sb[:, tt, :],
                x_bf.ap().rearrange("(n p) d -> n p d", p=P)[tt])
            nc.vector.tensor_reduce(musb[:, tt:tt + 1], x_sb[:, tt, :],
                                    AX.X, ALU.add)
            sq = p1s.tile([128, D], BF16, name="sq")
            nc.scalar.activation(sq[:], x_sb[:, tt, :], AF.Square,
                                 accum_out=sssb[:, tt:tt + 1])
        mu = p1.tile([128, TT], F32, name="mu")
        nc.vector.tensor_scalar_mul(mu[:], musb[:], 1.0 / D)
        var = p1.tile([128, TT], F32, name="var")
        nc.vector.tensor_tensor(var[:], mu[:], mu[:], ALU.mult)
        ex2 = p1.tile([128, TT], F32, name="ex2")
        nc.vector.tensor_scalar_mul(ex2[:], sssb[:], 1.0 / D)
        nc.vector.tensor_tensor(var[:], ex2[:], var[:], ALU.subtract)
        sd = p1.tile([128, TT], F32, name="sd")
        nc.scalar.activation(sd[:], var[:], AF.Sqrt, bias=eps_sb[:])
        nc.vector.reciprocal(stats[:, 0, :], sd[:])
        nc.vector.tensor_tensor(stats[:, 1, :], stats[:, 0, :], mu[:],
                                ALU.mult)
        nc.vector.tensor_scalar_mul(stats[:, 1, :], stats[:, 1, :], -1.0)
        for tt in range(TT):
            g, i = tt // (TT // 2), tt % (TT // 2)
            ht = p1.tile([128, D], BF16, name="ht")
            nc.scalar.activation(ht[:], x_sb[:, tt, :], AF.Identity,
                                 bias=stats[:, 1, tt:tt + 1],
                                 scale=stats[:, 0, tt:tt + 1])
            nc.sync.dma_start(
                h_dram[g][:].rearrange("(n p) d -> n p d", p=P)[i], ht[:])

    p0.release()
    # =====================================================================
    # P2: transpose-load hT, projections
    # =====================================================================
    p2h = tc.alloc_tile_pool(name="p2h", bufs=1)
    hT = p2h.tile([128, DT, TOK], BF16, name="hT")
    for g in range(2):
        for dt in range(DT):
            nc.sync.dma_start_transpose(
                hT[:, dt, g * (TOK // 2):(g + 1) * (TOK // 2)],
                h_dram[g][:, dt * 128:(dt + 1) * 128])

    with tc.tile_pool(name="proj_ps", bufs=2, space="PSUM") as pps:
        for tci in range(4):
            tcsl = slice(tci * 512, (tci + 1) * 512)
            for (dst, w_sb, bias, fn) in (
                    (qT, wq_sb, bqp_sb, AF.Silu),
                    (kT, wk_sb, bkp_sb, AF.Silu),
                    (vT, wv_sb, bvp_sb, AF.Identity),
                    (fT, wf_sb, bfp_sb, AF.Identity)):
                bank = pps.tile([128, 512], F32, name="projbank")
                for dt in range(DT):
                    nc.tensor.matmul(bank[:], w_sb[:, dt, :], hT[:, dt, tcsl],
                                     start=(dt == 0), stop=(dt == DT - 1))
                nc.scalar.activation(dst[:, tcsl], bank[:], fn,
                                     bias=bias[:], scale=1.0)

    p2h.release()
    # roundtrip k/v/f to token-major via DMA transpose
    nc.sync.dma_start(k_dram[:], kT[:])
    nc.sync.dma_start(v_dram[:], vT[:])
    nc.sync.dma_start(f_dram[:], fT[:])
    for tt in range(TT):
        ttsl = slice(tt * 128, (tt + 1) * 128)
        nc.sync.dma_start_transpose(k_tm[:, tt, :], k_dram[:, ttsl])
        nc.sync.dma_start_transpose(v_tm[:, tt, :], v_dram[:, ttsl])
        nc.sync.dma_start_transpose(f_tm[:, tt, :], f_dram[:, ttsl])

    with tc.tile_pool(name="sgate", bufs=2) as sg:
        for tci in range(4):
            csl = slice(tci * 4, (tci + 1) * 4)
            enf = sg.tile([128, 4, 128], F32, name="enf")
            nc.scalar.activation(enf[:], f_tm[:, csl, :], AF.Exp,
                                 scale=-1.0)
            nc.scalar.activation(sp[:, csl, :], enf[:], AF.Ln, bias=1.0)
            e8 = sg.tile([128, 4, 128], BF16, name="e8")
            nc.scalar.activation(e8[:], sp[:, csl, :], AF.Exp, scale=-0.125)
            nc.vector.tensor_scalar(s_tm[:, csl, :],
                                    e8[:], -1.0, 1.0, ALU.mult, ALU.add)

    for nm, t_sb in (("qT", qT), ("kT", kT)):
        if (d := dump(nm, [128, TOK], BF16)) is not None:
            nc.sync.dma_start(d.ap(), t_sb[:])
    for nm, t_sb in (("k_tm", k_tm), ("v_tm", v_tm), ("s_tm", s_tm)):
        if (d := dump(nm, [128, TT * 128], BF16)) is not None:
            nc.sync.dma_start(d.ap().rearrange("p (n f) -> p n f", n=TT),
                              t_sb[:])
    if (d := dump("sp", [128, TT * 128])) is not None:
        nc.sync.dma_start(d.ap().rearrange("p (n f) -> p n f", n=TT), sp[:])

    # =====================================================================
    # P3: chunked scan, b-major; RMS + write o_nT
    # =====================================================================
    with tc.tile_pool(name="scan_ps", bufs=2, space="PSUM") as sps, \
         tc.tile_pool(name="scan_sb", bufs=2) as ssb, \
         tc.tile_pool(name="state_sb", bufs=1) as stb:
        Kst = stb.tile([128, 64], BF16, name="Kst")   # [2h*64 dk, s]
        Vst = stb.tile([128, 64], BF16, name="Vst")   # [2h*64 s, dv]
        for b in range(B):
            for c in range(NCH):
                bi = b * 4 + c
                tsl = slice(b * 512 + c * 128, b * 512 + (c + 1) * 128)
                first = (c == 0)
                bankA = sps.tile([128, 512], F32, name="bankA", bufs=1)
                ps_b = bankA[:, 0:128]
                ps_ss = bankA[:, 128:256]
                ps_lc = bankA[:, 256:257]
                ps_lcr = bankA[0:1, 257:385]
                ps_lambc = bankA[:, 384:512]
                bankB = sps.tile([128, 512], F32, name="bankB")
                ps_a = (bankB[:, 0:128], bankB[:, 128:256])
                ps_ok = bankB[:, 256:384]
                bankD = sps.tile([128, 1024], BF16, name="bankD", bufs=1)
                ps_pt = (bankD[0:64, 0:128], bankD[0:64, 128:256])
                ps_st = (bankD[0:64, 256:384], bankD[0:64, 384:512])
                bankE = sps.tile([128, 512], F32, name="bankE", bufs=1)
                ps_b2 = (bankE[:, 0:128], bankE[:, 128:256])
                bankF = sps.tile([128, 512], F32, name="bankF")
                ps_o = (bankF[0:64, 0:128], bankF[0:64, 128:256])
                ps_dk = (bankF[0:64, 256:320], bankF[0:64, 320:384])
                ps_dv = (bankF[0:64, 384:448], bankF[0:64, 448:512])

                # cumsum b = ltriT.T @ sp (f32); colsum -> bCT
                nc.tensor.matmul(ps_b, ltriT_sb[:], sp[:, bi, :],
                                 start=True, stop=True)
                nc.tensor.matmul(ps_lc, sp[:, bi, :], onescol_sb[:],
                                 start=True, stop=True)
                lam = ssb.tile([128, 128], BF16, name="lam")
                nc.scalar.activation(lam[:], ps_b, AF.Exp)
                lam_s = ssb.tile([128, 128], BF16, name="lam_s")
                nc.vector.tensor_scalar_mul(lam_s[:], lam[:], 0.125)
                en = ssb.tile([128, 128], BF16, name="en")
                with nc.allow_low_precision(reason="en=1/lam feeds bf16"):
                    nc.vector.reciprocal(en[:], lam[:])
                lamCT = ssb.tile([128, 1], F32, name="lamCT")
                nc.scalar.activation(lamCT[:], ps_lc, AF.Exp)
                nc.tensor.matmul(ps_lcr, onescol_sb[:], sp[:, bi, :],
                                 start=True, stop=True)
                lamCr = ssb.tile([1, 128], BF16, name="lamCr")
                nc.scalar.activation(lamCr[:], ps_lcr, AF.Exp)

                s_til = ssb.tile([128, 128], BF16, name="s_til")
                nc.vector.tensor_tensor(s_til[:], s_tm[:, bi, :], en[:],
                                        ALU.mult)
                nc.tensor.matmul(ps_lambc, ones_row_sb[:], lamCr[:],
                                 start=True, stop=True)
                s2 = ssb.tile([128, 128], BF16, name="s2")
                nc.vector.tensor_tensor(s2[:], s_til[:], ps_lambc, ALU.mult)

                am = ssb.tile([128, 256], BF16, name="am")
                for h in range(2):
                    hs = slice(h * 64, (h + 1) * 64)
                    nc.tensor.matmul(ps_a[h], kT[hs, tsl], qT[hs, tsl],
                                     start=True, stop=True)
                    nc.vector.tensor_tensor(am[:, h * 128:(h + 1) * 128],
                                            ps_a[h], cmask_sb[:], ALU.mult)
                for h in range(2):
                    hs = slice(h * 64, (h + 1) * 64)
                    oks = ps_ok[:, h * 64:(h + 1) * 64]
                    if not first:
                        nc.tensor.matmul(oks, qT[hs, tsl], Kst[hs, :],
                                         start=True, stop=False)
                    nc.tensor.matmul(oks, am[:, h * 128:(h + 1) * 128],
                                     s_til[:, h * 64:(h + 1) * 64],
                                     start=first, stop=True)
                # softmax over slots (per head), pl = P * lam
                oksc = ssb.tile([128, 128], F32, name="oksc")
                nc.vector.tensor_tensor(oksc[:], ps_ok, lam_s[:], ALU.mult)
                ex = ssb.tile([128, 128], BF16, name="ex")
                nc.scalar.activation(ex[:], oksc[:], AF.Exp)
                rsum = ssb.tile([128, 2], F32, name="rsum")
                nc.vector.tensor_reduce(
                    rsum[:], ex[:].rearrange("p (h s) -> p h s", h=2),
                    AX.X, ALU.add)
                rcp = ssb.tile([128, 2], F32, name="rcp")
                nc.vector.reciprocal(rcp[:], rsum[:])
                pl = ssb.tile([128, 128], BF16, name="pl")
                nc.vector.tensor_tensor(pl[:], ex[:], lam[:], ALU.mult)
                nc.vector.tensor_tensor(
                    pl[:].rearrange("p (h s) -> p h s", h=2),
                    pl[:].rearrange("p (h s) -> p h s", h=2),
                    rcp[:].rearrange("p (h o) -> p h o", h=2)
                        .to_broadcast([128, 2, 64]),
                    ALU.mult)

                # transposes: plT, s_tilT  [2h*64 s, 128 t]
                plT = ssb.tile([128, 128], BF16, name="plT")
                s_tilT = ssb.tile([128, 128], BF16, name="s_tilT")
                for h in range(2):
                    hs = slice(h * 64, (h + 1) * 64)
                    nc.tensor.transpose(ps_pt[h], pl[:, hs], ident_sb[:])
                    nc.vector.tensor_copy(plT[hs, :], ps_pt[h])
                    nc.tensor.transpose(ps_st[h], s_til[:, hs], ident_sb[:])
                    nc.vector.tensor_copy(s_tilT[hs, :], ps_st[h])

                b2m = ssb.tile([128, 256], BF16, name="b2m")
                for h in range(2):
                    hs = slice(h * 64, (h + 1) * 64)
                    nc.tensor.matmul(ps_b2[h], s_tilT[hs, :], plT[hs, :],
                                     start=True, stop=True)
                    nc.vector.tensor_tensor(b2m[:, h * 128:(h + 1) * 128],
                                            ps_b2[h], cmask_sb[:], ALU.mult)
                for h in range(2):
                    hs = slice(h * 64, (h + 1) * 64)
                    if not first:
                        nc.tensor.matmul(ps_o[h], Vst[hs, :], plT[hs, :],
                                         start=True, stop=False)
                    nc.tensor.matmul(ps_o[h], v_tm[:, bi, hs],
                                     b2m[:, h * 128:(h + 1) * 128],
                                     start=first, stop=True)
                    nc.tensor.matmul(ps_dk[h], k_tm[:, bi, hs],
                                     s2[:, hs], start=True, stop=True)
                    nc.tensor.matmul(ps_dv[h], s2[:, hs], v_tm[:, bi, hs],
                                     start=True, stop=True)
                    if first:
                        nc.vector.tensor_copy(Kst[hs, :], ps_dk[h])
                        nc.vector.tensor_copy(Vst[hs, :], ps_dv[h])
                    else:
                        nc.vector.tensor_tensor(
                            Kst[hs, :], Kst[hs, :],
                            ps_lambc[hs, hs], ALU.mult)
                        nc.vector.tensor_tensor(Kst[hs, :], Kst[hs, :],
                                                ps_dk[h], ALU.add)
                        nc.vector.tensor_scalar(Vst[hs, :], Vst[hs, :],
                                                lamCT[hs, 0:1], None, ALU.mult)
                        nc.vector.tensor_tensor(Vst[hs, :], Vst[hs, :],
                                                ps_dv[h], ALU.add)

                # write raw oT (RMS batched after the loop)
                nc.vector.tensor_copy(onT[0:64, tsl], ps_o[0])
                nc.vector.tensor_copy(onT[64:128, tsl], ps_o[1])

    # batched RMS over dv for the whole oT
    with tc.tile_pool(name="rms_ps", bufs=2, space="PSUM") as rps, \
         tc.tile_pool(name="rms_sb", bufs=2) as rsb:
        for q4 in range(4):
            qsl = slice(q4 * 512, (q4 + 1) * 512)
            sqo = rsb.tile([128, 512], BF16, name="sqo")
            nc.vector.tensor_tensor(sqo[:], onT[:, qsl], onT[:, qsl],
                                    ALU.mult)
            ps_ss = rps.tile([128, 512], F32, name="ps_ss")
            nc.tensor.matmul(ps_ss[:], bd128_sb[:], sqo[:],
                             start=True, stop=True)
            sdo = rsb.tile([128, 512], F32, name="sdo")
            nc.scalar.activation(sdo[:], ps_ss[:], AF.Sqrt,
                                 bias=eps_sb[:], scale=1.0 / DV)
            rro = rsb.tile([128, 512], F32, name="rro")
            nc.vector.reciprocal(rro[:], sdo[:])
            nc.vector.tensor_tensor(onT[:, qsl], onT[:, qsl], rro[:],
                                    ALU.mult)

    if (d := dump("onT", [128, TOK], BF16)) is not None:
        nc.sync.dma_start(d.ap(), onT[:])

    # head-sharded -> token-sharded redistribution
    nc.sync.dma_start(
        a2a_in[:].rearrange("(r p) t -> p r t", p=P),
        onT[:].rearrange("p (r t) -> p r t", r=N_CORES))
    nc.gpsimd.collective_compute("AllToAll", ALU.bypass, replica_groups=RG,
                                 ins=[a2a_in.opt()], outs=[a2a_out.opt()])

    # =====================================================================
    # P4 tail: out-proj + residual + LN2 + MLP on 256 tokens
    # =====================================================================
    with tc.tile_pool(name="tail_ps", bufs=1, space="PSUM") as tps, \
         tc.tile_pool(name="tail_sb", bufs=2) as tsb, \
         tc.tile_pool(name="tail_keep", bufs=1) as tkb, \
         tc.tile_pool(name="w1stream", bufs=3) as w1s, \
         tc.tile_pool(name="w2stream", bufs=4) as w2s:
        ofT = tkb.tile([128, DT, TAIL], BF16, name="ofT")
        nc.sync.dma_start(ofT[:],
                          a2a_out[:].rearrange("(jt p) t -> p jt t", p=P))
        x2 = tkb.tile([128, 2, D], F32, name="x2")
        nc.sync.dma_start(x2[:],
                          x_res.ap().rearrange("(n p) d -> p n d", p=P))

        op_bank = tps.tile([128, 512], F32, name="op_bank")
        for tt2 in range(2):
            for nb in range(2):
                nsl = slice(nb * 512, (nb + 1) * 512)
                for jt in range(DT):
                    nc.tensor.matmul(op_bank[:],
                                     ofT[:, jt, tt2 * 128:(tt2 + 1) * 128],
                                     wo_sb[:, jt, nsl],
                                     start=(jt == 0), stop=(jt == DT - 1))
                nc.vector.tensor_tensor(x2[:, tt2, nsl], op_bank[:],
                                        x2[:, tt2, nsl], ALU.add)
        if (d := dump("x2", [128, 2 * D])) is not None:
            nc.sync.dma_start(d.ap().rearrange("p (n f) -> p n f", n=2),
                              x2[:])

        # LN2 + store h2, transpose-load
        h2T = tkb.tile([128, DT, TAIL], BF16, name="h2T")
        for tt2 in range(2):
            x2t = x2[:, tt2, :]
            ssum = tsb.tile([128, 1], F32, name="ssum2")
            nc.vector.tensor_reduce(ssum[:], x2t, AX.X, ALU.add)
            sq = tsb.tile([128, D], BF16, name="sq2")
            ssq = tsb.tile([128, 1], F32, name="ssq2")
            nc.scalar.activation(sq[:], x2t, AF.Square, accum_out=ssq[:])
            mu = tsb.tile([128, 1], F32, name="mu2")
            nc.vector.tensor_scalar_mul(mu[:], ssum[:], 1.0 / D)
            var = tsb.tile([128, 1], F32, name="var2")
            nc.vector.tensor_tensor(var[:], mu[:], mu[:], ALU.mult)
            ex2 = tsb.tile([128, 1], F32, name="ex22")
            nc.vector.tensor_scalar_mul(ex2[:], ssq[:], 1.0 / D)
            nc.vector.tensor_tensor(var[:], ex2[:], var[:], ALU.subtract)
            sd = tsb.tile([128, 1], F32, name="sd2")
            nc.scalar.activation(sd[:], var[:], AF.Sqrt, bias=eps_sb[:])
            r2 = tsb.tile([128, 1], F32, name="r2")
            nc.vector.reciprocal(r2[:], sd[:])
            nrmu = tsb.tile([128, 1], F32, name="nrmu2")
            nc.vector.tensor_tensor(nrmu[:], r2[:], mu[:], ALU.mult)
            nc.vector.tensor_scalar_mul(nrmu[:], nrmu[:], -1.0)
            h2t = tsb.tile([128, D], BF16, name="h2t")
            nc.scalar.activation(h2t[:], x2t, AF.Identity,
                                 bias=nrmu[:], scale=r2[:])
            nc.sync.dma_start(
                h2d[:].rearrange("(n p) d -> n p d", p=P)[tt2], h2t[:])
        for dt in range(DT):
            nc.sync.dma_start_transpose(h2T[:, dt, :],
                                        h2d[:, dt * 128:(dt + 1) * 128])

        # MLP1: y1 token-major [t, mlp-chunk], gelu, roundtrip to zT
        z_tm = tkb.tile([128, 2, MLP], BF16, name="z_tm")
        for mc in range(8):
            mcsl = slice(mc * 512, (mc + 1) * 512)
            w1t = w1s.tile([128, DT, 512], BF16, name="w1t")
            nc.sync.dma_start(
                w1t[:], w1.ap().rearrange("(dt p) m -> p dt m", p=P)
                [:, :, mcsl])
            for tt2 in range(2):
                y1b = tps.tile([128, 512], F32, name="y1b", bufs=2)
                for dt in range(DT):
                    nc.tensor.matmul(y1b[:],
                                     h2T[:, dt, tt2 * 128:(tt2 + 1) * 128],
                                     w1t[:, dt, :],
                                     start=(dt == 0), stop=False)
                nc.tensor.matmul(y1b[:], ones_row_sb[:], b1row_sb[:, mcsl],
                                 start=False, stop=True)
                nc.scalar.activation(z_tm[:, tt2, mcsl], y1b[:], AF.Gelu)
        nc.sync.dma_start(
            z_dram[:].rearrange("(n p) m -> p n m", p=P), z_tm[:])
        zT = tkb.tile([128, MT, TAIL], BF16, name="zT")
        for mt in range(MT):
            nc.sync.dma_start_transpose(
                zT[:, mt, :], z_dram[:, mt * 128:(mt + 1) * 128])

        # MLP2: y2 = z @ w2, accumulate over mt into 4 resident banks
        y2_banks = [tps.tile([128, 512], F32, name=f"y2b{i}")
                    for i in range(4)]
        for mt in range(MT):
            w2t = w2s.tile([128, D], BF16, name="w2t")
            nc.sync.dma_start(
                w2t[:], w2.ap().rearrange("(n p) d -> n p d", p=P)[mt])
            for tt2 in range(2):
                for nb in range(2):
                    nc.tensor.matmul(
                        y2_banks[tt2 * 2 + nb],
                        zT[:, mt, tt2 * 128:(tt2 + 1) * 128],
                        w2t[:, nb * 512:(nb + 1) * 512],
                        start=(mt == 0), stop=(mt == MT - 1))
        for tt2 in range(2):
            for nb in range(2):
                nsl = slice(nb * 512, (nb + 1) * 512)
                ys = tsb.tile([128, 512], F32, name="ys")
                nc.vector.tensor_tensor(ys[:], y2_banks[tt2 * 2 + nb],
                                        x2[:, tt2, nsl], ALU.add)
                nc.sync.dma_start(
                    y_out.ap().rearrange("(n p) d -> p n d", p=P)
                    [:, tt2, nsl], ys[:])

    for pool in (dram, persist, const):
        pool.release()


def _build():
    nc = bacc.Bacc("TRN2", target_bir_lowering=False, debug=False,
                   num_devices=N_CORES)

    def din(name, shape, dt=BF16):
        return nc.dram_tensor(name, shape, dt, kind="ExternalInput")

    io = dict(
        x_bf=din("x_bf", [TOK, D]),
        x_res=din("x_res", [TAIL, D], F32),
        wq=din("wq", [D, 128]), wk=din("wk", [D, 128]),
        wv=din("wv", [D, 128]), wf=din("wf", [D, 128]),
        bqp=din("bqp", [128, 1], F32), bkp=din("bkp", [128, 1], F32),
        bvp=din("bvp", [128, 1], F32), bfp=din("bfp", [128, 1], F32),
        b1row=din("b1row", [1, MLP]),
        wo=din("wo", [D, D]),
        w1=din("w1", [D, MLP]),
        b1v=din("b1v", [128, MLP // 128], F32),
        w2=din("w2", [MLP, D]),
        b2v=din("b2v", [1, D], F32),
        ltriT=din("ltriT", [128, 128], F32),
        onescol=din("onescol", [128, 1], F32),
        cmask=din("cmask", [128, 128]),
        ident=din("ident", [128, 128]),
        bd128=din("bd128", [128, 128]),
        ones_row=din("ones_row", [1, 128]),
        y_out=nc.dram_tensor("y_out", [TAIL, D], F32, kind="ExternalOutput"),
    )

    dbg = [s for s in os.environ.get("GSA_DEBUG", "").split(",") if s]
    dbg_outs = {}

    def dump(name, shape, dt=F32):
        if name in dbg:
            t = nc.dram_tensor("dbg_" + name, shape, dt,
                               kind="ExternalOutput")
            dbg_outs[name] = t
            return t
        return None

    io["dump"] = dump
    with tile.TileContext(nc) as tcx:
        _emit(nc, tcx, io)
    nc.compile()
    return nc, sorted(dbg_outs)


def _host_prep(inputs):
    """Fold norms/biases into weights; build per-core in_maps."""
    f32 = np.float32
    bf16 = ml_dtypes.bfloat16
    x = np.asarray(inputs["hidden_states"], f32).reshape(TOK, D)
    ln1_w = np.asarray(inputs["ln1_w"], f32)
    ln1_b = np.asarray(inputs["ln1_b"], f32)
    ln2_w = np.asarray(inputs["ln2_w"], f32)
    ln2_b = np.asarray(inputs["ln2_b"], f32)
    gnorm = np.asarray(inputs["gnorm_w"], f32)
    Wq = np.asarray(inputs["Wq"], f32) * ln1_w[:, None]
    Wk = np.asarray(inputs["Wk"], f32) * ln1_w[:, None]
    Wv = np.asarray(inputs["Wv"], f32) * ln1_w[:, None]
    Wf = np.asarray(inputs["Wf"], f32) * ln1_w[:, None]
    bq = ln1_b @ np.asarray(inputs["Wq"], f32)
    bk = ln1_b @ np.asarray(inputs["Wk"], f32)
    bv = ln1_b @ np.asarray(inputs["Wv"], f32)
    bf_ = ln1_b @ np.asarray(inputs["Wf"], f32)
    Wo = np.asarray(inputs["Wo"], f32) * np.tile(gnorm, H)[:, None]
    W1 = np.asarray(inputs["W1"], f32) * ln2_w[:, None]
    b1 = np.asarray(inputs["b1"], f32) + ln2_b @ np.asarray(inputs["W1"], f32)
    W2 = np.asarray(inputs["W2"], f32)
    b2 = np.asarray(inputs["b2"], f32)

    tri = np.tril(np.ones((128, 128), f32))  # [t, tau] tau<=t
    ltriT = np.ascontiguousarray((-0.125 * tri).T)           # [tau, t]
    cmask = np.ascontiguousarray(tri.T.astype(bf16))         # [tau, t]
    ident = np.eye(128, dtype=bf16)
    bd128 = np.kron(np.eye(2, dtype=f32),
                    np.ones((64, 64), f32)).astype(bf16)
    common = dict(
        x_bf=np.ascontiguousarray(x.astype(bf16)),
        ltriT=ltriT,
        onescol=np.full((128, 1), -0.125, f32),
        cmask=cmask, ident=ident, bd128=bd128,
        ones_row=np.ones((1, 128), bf16),
        wo=np.ascontiguousarray(Wo.astype(bf16)),
        w1=np.ascontiguousarray(W1.astype(bf16)),
        b1v=np.ascontiguousarray(b1.reshape(MLP // 128, 128).T.astype(f32)),
        b1row=np.ascontiguousarray(b1.reshape(1, MLP).astype(bf16)),
        w2=np.ascontiguousarray(W2.astype(bf16)),
        b2v=np.ascontiguousarray(b2.reshape(1, D)),
    )
    in_maps = []
    for r in range(N_CORES):
        jsl = slice(r * 128, (r + 1) * 128)  # 2 heads = 128 cols
        m = dict(common)
        m["x_res"] = np.ascontiguousarray(x[r * TAIL:(r + 1) * TAIL]
                                          + b2[None, :])
        m["wq"] = np.ascontiguousarray(Wq[:, jsl].astype(bf16))
        m["wk"] = np.ascontiguousarray(Wk[:, jsl].astype(bf16))
        m["wv"] = np.ascontiguousarray(Wv[:, jsl].astype(bf16))
        m["wf"] = np.ascontiguousarray(Wf[:, jsl].astype(bf16))
        m["bqp"] = np.ascontiguousarray(bq[jsl].reshape(128, 1))
        m["bkp"] = np.ascontiguousarray(bk[jsl].reshape(128, 1))
        m["bvp"] = np.ascontiguousarray(bv[jsl].reshape(128, 1))
        m["bfp"] = np.ascontiguousarray(bf_[jsl].reshape(128, 1))
        in_maps.append(m)
    return in_maps


def kernel(**inputs):
    if "nc" not in _cache:
        _cache["nc"], _cache["dbg"] = _build()
    nc = _cache["nc"]
    in_maps = _host_prep(inputs)
    res = run_bass_kernel_spmd(nc, in_maps, core_ids=list(range(N_CORES)),
                               trace=bool(os.environ.get("GSA_TRACE")))
    _cache["last_results"] = res
    out = np.concatenate([res.results[r]["y_out"] for r in range(N_CORES)],
                         axis=0)
    return out.reshape(B, T, D)



# revision 11
# speedup vs baseline: 1.3481x; 1.3481x over previous
"""GSA video block kernel for 8 TRN2 NeuronCores.

Sharding: head-parallel attention (2 heads/core) -> one AllToAll that
redistributes the RMS-normed head outputs from head-sharded to
token-sharded -> token-parallel tail (out-proj + LN2 + MLP with full
weights, 256 tokens/core).

The sequential T=512 gated-slot-attention scan is replaced by an exact
chunk-parallel formulation (C=128): intra-chunk terms via causal-masked
matmuls with per-slot decay factors, inter-chunk via carried states
K[DK,M] / V[M,DV].

v2: all layout changes (h, k/v/f, h2, z transposes) are done with PE
transposes + PSUM copies instead of DRAM-roundtrip DMA transposes;
MLP1 produces zT directly; rsqrt via Ln/Exp keeps one activation table
loaded through the scan; elementwise work is spread over
vector/scalar/gpsimd.
"""

import os
import sys

import numpy as np
import ml_dtypes

if "/opt/trn_rl_repo" not in sys.path:
    sys.path.insert(0, "/opt/trn_rl_repo")

import concourse.bass as bass  # noqa: E402
import concourse.mybir as mybir  # noqa: E402
import concourse.tile as tile  # noqa: E402
from concourse import bacc  # noqa: E402
from concourse.bass_utils import run_bass_kernel_spmd  # noqa: E402

BF16 = mybir.dt.bfloat16
F32 = mybir.dt.float32
AF = mybir.ActivationFunctionType
ALU = mybir.AluOpType
AX = mybir.AxisListType

B, T, D = 4, 512, 1024
H, DK, DV, M = 16, 64, 64, 64
MLP = 4096
EPS = 1e-6

N_CORES = 8
C = 128                    # scan chunk length
NCH = T // C               # chunks per batch = 4
TOK = B * T                # 2048 flat tokens
TT = TOK // 128            # 16 token tiles
DT = D // 128              # 8 d tiles
MT = MLP // 128            # 32 mlp tiles
TAIL = TOK // N_CORES      # 256 tokens per core in the tail
LN8 = float(np.log(0.125))
RG = [list(range(N_CORES))]

_cache = {}


def _emit(nc, tc, io):
    x_bf, x_res = io["x_bf"], io["x_res"]
    wq, wk, wv, wf = io["wq"], io["wk"], io["wv"], io["wf"]
    bqp, bkp = io["bqp"], io["bkp"]
    bvp, bfp = io["bvp"], io["bfp"]
    wo, w1, b1v, w2 = io["wo"], io["w1"], io["b1v"], io["w2"]
    ltriT, onescol, cmask = io["ltriT"], io["onescol"], io["cmask"]
    ident, bd128, ones_row = io["ident"], io["bd128"], io["ones_row"]
    y_out, dump = io["y_out"], io["dump"]
    P = 128

    const = tc.alloc_tile_pool(name="const", bufs=1)
    persist = tc.alloc_tile_pool(name="persist", bufs=1)
    dram = tc.alloc_tile_pool(name="dram", bufs=1, space="DRAM")

    # ---- warmup collective (prepay ncfw handshake) -----------------------
    wa_in = dram.tile([8, 128], BF16, name="wa_in")
    wa_out = dram.tile([8, 128], BF16, name="wa_out")
    nc.gpsimd.collective_compute("AllReduce", ALU.add, replica_groups=RG,
                                 ins=[wa_in.opt()], outs=[wa_out.opt()])

    # ---- constants into SBUF --------------------------------------------
    def cload(ap, shape, dt, name):
        t = const.tile(shape, dt, name=name)
        nc.sync.dma_start(t[:], ap)
        return t

    ltriT_sb = cload(ltriT.ap(), [128, 128], F32, "ltriT")
    onescol_sb = cload(onescol.ap(), [128, 1], F32, "onescol")
    cmask_sb = cload(cmask.ap(), [128, 128], BF16, "cmask")
    ident_sb = cload(ident.ap(), [128, 128], BF16, "ident")
    bd128_sb = cload(bd128.ap(), [128, 128], BF16, "bd128")
    ones_row_sb = cload(ones_row.ap(), [1, 128], BF16, "ones_row")
    bqp_sb = cload(bqp.ap(), [128, 1], F32, "bqp")
    bkp_sb = cload(bkp.ap(), [128, 1], F32, "bkp")
    bvp_sb = cload(bvp.ap(), [128, 1], F32, "bvp")
    bfp_sb = cload(bfp.ap(), [128, 1], F32, "bfp")
    b1_sb = cload(b1v.ap(), [128, MT], F32, "b1")
    eps_sb = const.tile([128, 1], F32)
    nc.vector.memset(eps_sb[:], EPS)
    ln8_sb = const.tile([128, 1], F32)
    nc.vector.memset(ln8_sb[:], LN8)

    wq_sb = const.tile([128, DT, 128], BF16)
    nc.sync.dma_start(wq_sb[:], wq.ap().rearrange("(dt p) j -> p dt j", p=P))
    wk_sb = const.tile([128, DT, 128], BF16)
    nc.sync.dma_start(wk_sb[:], wk.ap().rearrange("(dt p) j -> p dt j", p=P))
    wv_sb = const.tile([128, DT, 128], BF16)
    nc.sync.dma_start(wv_sb[:], wv.ap().rearrange("(dt p) j -> p dt j", p=P))
    wf_sb = const.tile([128, DT, 128], BF16)
    nc.sync.dma_start(wf_sb[:], wf.ap().rearrange("(dt p) j -> p dt j", p=P))
    wo_sb = const.tile([128, DT, D], BF16)
    nc.sync.dma_start(wo_sb[:], wo.ap().rearrange("(dt p) j -> p dt j", p=P))

    # ---- persistent activation tensors ----------------------------------
    qT = persist.tile([128, TOK], BF16, name="qT")       # [2h*64 dk, t]
    kT = persist.tile([128, TOK], BF16, name="kT")
    vT = persist.tile([128, TOK], BF16, name="vT")
    fT = persist.tile([128, TOK], BF16, name="fT")
    k_tm = persist.tile([128, TT, 128], BF16, name="k_tm")   # [t, 2h*64]
    v_tm = persist.tile([128, TT, 128], BF16, name="v_tm")
    f_tm = persist.tile([128, TT, 128], BF16, name="f_tm")
    sp = persist.tile([128, TT, 128], F32, name="sp")        # softplus(-f)
    s_tm = persist.tile([128, TT, 128], BF16, name="s_tm")   # 1-exp(g)
    onT = persist.tile([128, TOK], BF16, name="onT")         # normed oT

    a2a_in = dram.tile([128 * N_CORES, TAIL], BF16, name="a2a_in")
    a2a_out = dram.tile([128 * N_CORES, TAIL], BF16, name="a2a_out")

    p2h = tc.alloc_tile_pool(name="p2h", bufs=1)
    hT = p2h.tile([128, DT, TOK], BF16, name="hT")

    # =====================================================================
    # P1: LN1 stats (batched), then per-tc: normalize + PE-transpose to hT
    #     + q/k projections (Silu era)
    # =====================================================================
    p0 = tc.alloc_tile_pool(name="p0", bufs=1)
    x_sb = p0.tile([128, TT, D], BF16, name="x_sb")
    stats = p0.tile([128, 2, TT], F32, name="stats")  # [.,0,:]=r [.,1,:]=nrmu
    with tc.tile_pool(name="p1", bufs=3) as p1, \
         tc.tile_pool(name="p1s", bufs=2) as p1s, \
         tc.tile_pool(name="htp", bufs=3, space="PSUM") as htp, \
         tc.tile_pool(name="proj_ps", bufs=2, space="PSUM") as pps:
        musb = p1.tile([128, TT], F32, name="musb")
        sssb = p1.tile([128, TT], F32, name="sssb")
        for tt in range(TT):
            nc.sync.dma_start(
                x_sb[:, tt, :],
                x_bf.ap().rearrange("(n p) d -> n p d", p=P)[tt])
            nc.vector.tensor_reduce(musb[:, tt:tt + 1], x_sb[:, tt, :],
                                    AX.X, ALU.add)
            sq = p1s.tile([128, D], BF16, name="sq")
            nc.scalar.activation(sq[:], x_sb[:, tt, :], AF.Square,
                                 accum_out=sssb[:, tt:tt + 1])
        mu = p1.tile([128, TT], F32, name="mu")
        nc.vector.tensor_scalar_mul(mu[:], musb[:], 1.0 / D)
        var = p1.tile([128, TT], F32, name="var")
        nc.vector.tensor_tensor(var[:], mu[:], mu[:], ALU.mult)
        ex2 = p1.tile([128, TT], F32, name="ex2")
        nc.vector.tensor_scalar_mul(ex2[:], sssb[:], 1.0 / D)
        nc.vector.tensor_tensor(var[:], ex2[:], var[:], ALU.subtract)
        lnv = p1.tile([128, TT], F32, name="lnv")
        nc.scalar.activation(lnv[:], var[:], AF.Ln, bias=eps_sb[:])
        nc.scalar.activation(stats[:, 0, :], lnv[:], AF.Exp, scale=-0.5)
        nc.vector.tensor_tensor(stats[:, 1, :], stats[:, 0, :], mu[:],
                                ALU.mult)
        nc.vector.tensor_scalar_mul(stats[:, 1, :], stats[:, 1, :], -1.0)

        def proj(dst, w_sb, bias, fn, tcsl):
            bank = pps.tile([128, 512], F32, name="projbank")
            for dt in range(DT):
                nc.tensor.matmul(bank[:], w_sb[:, dt, :], hT[:, dt, tcsl],
                                 start=(dt == 0), stop=(dt == DT - 1))
            nc.scalar.activation(dst[:, tcsl], bank[:], fn,
                                 bias=bias[:], scale=1.0)

        for tci in range(4):
            for tt in range(tci * 4, tci * 4 + 4):
                ht = p1.tile([128, D], BF16, name="ht")
                nc.scalar.activation(ht[:], x_sb[:, tt, :], AF.Identity,
                                     bias=stats[:, 1, tt:tt + 1],
                                     scale=stats[:, 0, tt:tt + 1])
                ttsl = slice(tt * 128, (tt + 1) * 128)
                for dh in range(2):
                    tp = htp.tile([128, 4, 128], BF16, name="htps")
                    for dj in range(4):
                        dt = dh * 4 + dj
                        nc.tensor.transpose(
                            tp[:, dj, :], ht[:, dt * 128:(dt + 1) * 128],
                            ident_sb[:])
                        nc.vector.tensor_copy(hT[:, dt, ttsl], tp[:, dj, :])
            tcsl = slice(tci * 512, (tci + 1) * 512)
            proj(qT, wq_sb, bqp_sb, AF.Silu, tcsl)
            proj(kT, wk_sb, bkp_sb, AF.Silu, tcsl)

        # ================================================================
        # P2: v/f projections + PE transposes to token-major + gates
        # ================================================================
        with tc.tile_pool(name="kvftp", bufs=3, space="PSUM") as kvftp, \
             tc.tile_pool(name="sgate", bufs=2) as sg:
            for tci in range(4):
                tcsl = slice(tci * 512, (tci + 1) * 512)
                proj(vT, wv_sb, bvp_sb, AF.Identity, tcsl)
                proj(fT, wf_sb, bfp_sb, AF.Identity, tcsl)
                for tt in range(tci * 4, tci * 4 + 4):
                    ttsl = slice(tt * 128, (tt + 1) * 128)
                    tp = kvftp.tile([128, 3, 128], BF16, name="kvfps")
                    nc.tensor.transpose(tp[:, 0, :], kT[:, ttsl], ident_sb[:])
                    nc.vector.tensor_copy(k_tm[:, tt, :], tp[:, 0, :])
                    nc.tensor.transpose(tp[:, 1, :], vT[:, ttsl], ident_sb[:])
                    nc.scalar.activation(v_tm[:, tt, :], tp[:, 1, :],
                                         AF.Identity)
                    nc.tensor.transpose(tp[:, 2, :], fT[:, ttsl], ident_sb[:])
                    nc.vector.tensor_copy(f_tm[:, tt, :], tp[:, 2, :])
                csl = slice(tci * 4, (tci + 1) * 4)
                enf = sg.tile([128, 4, 128], F32, name="enf")
                nc.scalar.activation(enf[:], f_tm[:, csl, :], AF.Exp,
                                     scale=-1.0)
                nc.scalar.activation(sp[:, csl, :], enf[:], AF.Ln, bias=1.0)
                e8 = sg.tile([128, 4, 128], BF16, name="e8")
                nc.scalar.activation(e8[:], sp[:, csl, :], AF.Exp,
                                     scale=-0.125)
                nc.vector.tensor_scalar(s_tm[:, csl, :],
                                        e8[:], -1.0, 1.0, ALU.mult, ALU.add)

    p0.release()
    p2h.release()

    for nm, t_sb in (("qT", qT), ("kT", kT)):
        if (d := dump(nm, [128, TOK], BF16)) is not None:
            nc.sync.dma_start(d.ap(), t_sb[:])
    for nm, t_sb in (("k_tm", k_tm), ("v_tm", v_tm), ("s_tm", s_tm)):
        if (d := dump(nm, [128, TT * 128], BF16)) is not None:
            nc.sync.dma_start(d.ap().rearrange("p (n f) -> p n f", n=TT),
                              t_sb[:])
    if (d := dump("sp", [128, TT * 128])) is not None:
        nc.sync.dma_start(d.ap().rearrange("p (n f) -> p n f", n=TT), sp[:])

    # =====================================================================
    # P3: chunked scan, b-major; RMS + write o_nT
    # =====================================================================
    with tc.tile_pool(name="scan_ps", bufs=2, space="PSUM") as sps, \
         tc.tile_pool(name="scan_sb", bufs=2) as ssb, \
         tc.tile_pool(name="state_sb", bufs=1) as stb:
        Kst = stb.tile([128, 64], BF16, name="Kst")   # [2h*64 dk, s]
        Vst = stb.tile([128, 64], BF16, name="Vst")   # [2h*64 s, dv]
        for b in range(B):
            for c in range(NCH):
                bi = b * 4 + c
                tsl = slice(b * 512 + c * 128, b * 512 + (c + 1) * 128)
                first = (c == 0)
                bankA = sps.tile([128, 512], F32, name="bankA", bufs=1)
                ps_b = bankA[:, 0:128]
                ps_lc = bankA[:, 256:257]
                ps_lcr = bankA[0:1, 257:385]
                ps_lambc = bankA[:, 384:512]
                bankB = sps.tile([128, 512], F32, name="bankB")
                ps_a = (bankB[:, 0:128], bankB[:, 128:256])
                ps_ok = bankB[:, 256:384]
                bankD = sps.tile([128, 1024], BF16, name="bankD", bufs=1)
                ps_pt = bankD[:, 0:128]
                ps_st = bankD[:, 128:256]
                bankE = sps.tile([128, 512], F32, name="bankE", bufs=1)
                ps_b2 = (bankE[:, 0:128], bankE[:, 128:256])
                bankF = sps.tile([128, 512], F32, name="bankF")
                ps_o = (bankF[0:64, 0:128], bankF[0:64, 128:256])
                ps_dk = (bankF[0:64, 256:320], bankF[0:64, 320:384])
                ps_dv = (bankF[0:64, 384:448], bankF[0:64, 448:512])

                # cumsum b = ltriT.T @ sp (f32); colsum -> bCT
                nc.tensor.matmul(ps_b, ltriT_sb[:], sp[:, bi, :],
                                 start=True, stop=True)
                nc.tensor.matmul(ps_lc, sp[:, bi, :], onescol_sb[:],
                                 start=True, stop=True)
                lam = ssb.tile([128, 128], BF16, name="lam")
                nc.scalar.activation(lam[:], ps_b, AF.Exp)
                lam_s = ssb.tile([128, 128], BF16, name="lam_s")
                nc.scalar.activation(lam_s[:], ps_b, AF.Exp, bias=ln8_sb[:])
                en = ssb.tile([128, 128], BF16, name="en")
                nc.scalar.activation(en[:], ps_b, AF.Exp, scale=-1.0)
                lamCT = ssb.tile([128, 1], F32, name="lamCT")
                nc.scalar.activation(lamCT[:], ps_lc, AF.Exp)
                nc.tensor.matmul(ps_lcr, onescol_sb[:], sp[:, bi, :],
                                 start=True, stop=True)
                lamCr = ssb.tile([1, 128], BF16, name="lamCr")
                nc.scalar.activation(lamCr[:], ps_lcr, AF.Exp)

                s_til = ssb.tile([128, 128], BF16, name="s_til")
                nc.vector.tensor_tensor(s_til[:], s_tm[:, bi, :], en[:],
                                        ALU.mult)
                nc.tensor.matmul(ps_lambc, ones_row_sb[:], lamCr[:],
                                 start=True, stop=True)
                s2 = ssb.tile([128, 128], BF16, name="s2")
                nc.vector.tensor_tensor(s2[:], s_til[:], ps_lambc, ALU.mult)

                am = ssb.tile([128, 256], BF16, name="am")
                for h in range(2):
                    hs = slice(h * 64, (h + 1) * 64)
                    nc.tensor.matmul(ps_a[h], kT[hs, tsl], qT[hs, tsl],
                                     start=True, stop=True)
                    nc.vector.tensor_tensor(am[:, h * 128:(h + 1) * 128],
                                            ps_a[h], cmask_sb[:], ALU.mult)
                for h in range(2):
                    hs = slice(h * 64, (h + 1) * 64)
                    oks = ps_ok[:, h * 64:(h + 1) * 64]
                    if not first:
                        nc.tensor.matmul(oks, qT[hs, tsl], Kst[hs, :],
                                         start=True, stop=False)
                    nc.tensor.matmul(oks, am[:, h * 128:(h + 1) * 128],
                                     s_til[:, h * 64:(h + 1) * 64],
                                     start=first, stop=True)
                # softmax over slots (per head), pl = P * lam
                oksc = ssb.tile([128, 128], F32, name="oksc")
                nc.vector.tensor_tensor(oksc[:], ps_ok, lam_s[:], ALU.mult)
                ex = ssb.tile([128, 128], BF16, name="ex")
                nc.scalar.activation(ex[:], oksc[:], AF.Exp)
                rsum = ssb.tile([128, 2], F32, name="rsum")
                nc.vector.tensor_reduce(
                    rsum[:], ex[:].rearrange("p (h s) -> p h s", h=2),
                    AX.X, ALU.add)
                rcp = ssb.tile([128, 2], F32, name="rcp")
                nc.vector.reciprocal(rcp[:], rsum[:])
                pl = ssb.tile([128, 128], BF16, name="pl")
                nc.gpsimd.tensor_tensor(pl[:], ex[:], lam[:], ALU.mult)
                nc.vector.tensor_tensor(
                    pl[:].rearrange("p (h s) -> p h s", h=2),
                    pl[:].rearrange("p (h s) -> p h s", h=2),
                    rcp[:].rearrange("p (h o) -> p h o", h=2)
                        .to_broadcast([128, 2, 64]),
                    ALU.mult)

                # transposes: plT, s_tilT  [2h*64 s, 128 t]
                plT = ssb.tile([128, 128], BF16, name="plT")
                s_tilT = ssb.tile([128, 128], BF16, name="s_tilT")
                nc.tensor.transpose(ps_pt, pl[:], ident_sb[:])
                nc.scalar.activation(plT[:], ps_pt, AF.Identity)
                nc.tensor.transpose(ps_st, s_til[:], ident_sb[:])
                nc.vector.tensor_copy(s_tilT[:], ps_st)

                b2m = ssb.tile([128, 256], BF16, name="b2m")
                for h in range(2):
                    hs = slice(h * 64, (h + 1) * 64)
                    nc.tensor.matmul(ps_b2[h], s_tilT[hs, :], plT[hs, :],
                                     start=True, stop=True)
                    nc.vector.tensor_tensor(b2m[:, h * 128:(h + 1) * 128],
                                            ps_b2[h], cmask_sb[:], ALU.mult)
                for h in range(2):
                    hs = slice(h * 64, (h + 1) * 64)
                    if not first:
                        nc.tensor.matmul(ps_o[h], Vst[hs, :], plT[hs, :],
                                         start=True, stop=False)
                    nc.tensor.matmul(ps_o[h], v_tm[:, bi, hs],
                                     b2m[:, h * 128:(h + 1) * 128],
                                     start=first, stop=True)
                    nc.tensor.matmul(ps_dk[h], k_tm[:, bi, hs],
                                     s2[:, hs], start=True, stop=True)
                    nc.tensor.matmul(ps_dv[h], s2[:, hs], v_tm[:, bi, hs],
                                     start=True, stop=True)
                    if first:
                        nc.vector.tensor_copy(Kst[hs, :], ps_dk[h])
                        nc.scalar.activation(Vst[hs, :], ps_dv[h],
                                             AF.Identity)
                    else:
                        nc.vector.tensor_tensor(
                            Kst[hs, :], Kst[hs, :],
                            ps_lambc[hs, hs], ALU.mult)
                        nc.vector.tensor_tensor(Kst[hs, :], Kst[hs, :],
                                                ps_dk[h], ALU.add)
                        nc.gpsimd.tensor_scalar(Vst[hs, :], Vst[hs, :],
                                                lamCT[hs, 0:1], None,
                                                ALU.mult)
                        nc.vector.tensor_tensor(Vst[hs, :], Vst[hs, :],
                                                ps_dv[h], ALU.add)

                # write raw oT (RMS batched after the loop)
                nc.scalar.activation(onT[0:64, tsl], ps_o[0], AF.Identity)
                nc.scalar.activation(onT[64:128, tsl], ps_o[1], AF.Identity)

    # batched RMS over dv for the whole oT (rsqrt via Ln/Exp, no table swap)
    with tc.tile_pool(name="rms_ps", bufs=2, space="PSUM") as rps, \
         tc.tile_pool(name="rms_sb", bufs=2) as rsb:
        for q4 in range(4):
            qsl = slice(q4 * 512, (q4 + 1) * 512)
            sqo = rsb.tile([128, 512], BF16, name="sqo")
            nc.vector.tensor_tensor(sqo[:], onT[:, qsl], onT[:, qsl],
                                    ALU.mult)
            ps_ss = rps.tile([128, 512], F32, name="ps_ss")
            nc.tensor.matmul(ps_ss[:], bd128_sb[:], sqo[:],
                             start=True, stop=True)
            sdo = rsb.tile([128, 512], F32, name="sdo")
            nc.scalar.activation(sdo[:], ps_ss[:], AF.Ln,
                                 bias=eps_sb[:], scale=1.0 / DV)
            rro = rsb.tile([128, 512], F32, name="rro")
            nc.scalar.activation(rro[:], sdo[:], AF.Exp, scale=-0.5)
            nc.vector.tensor_tensor(onT[:, qsl], onT[:, qsl], rro[:],
                                    ALU.mult)

    if (d := dump("onT", [128, TOK], BF16)) is not None:
        nc.sync.dma_start(d.ap(), onT[:])

    # head-sharded -> token-sharded redistribution
    nc.sync.dma_start(
        a2a_in[:].rearrange("(r p) t -> p r t", p=P),
        onT[:].rearrange("p (r t) -> p r t", r=N_CORES))
    nc.gpsimd.collective_compute("AllToAll", ALU.bypass, replica_groups=RG,
                                 ins=[a2a_in.opt()], outs=[a2a_out.opt()])

    # =====================================================================
    # P4 tail: out-proj + residual + LN2 + MLP on 256 tokens
    # =====================================================================
    with tc.tile_pool(name="tail_ps", bufs=1, space="PSUM") as tps, \
         tc.tile_pool(name="mlp_ps", bufs=2, space="PSUM") as mps, \
         tc.tile_pool(name="tail_sb", bufs=2) as tsb, \
         tc.tile_pool(name="tail_keep", bufs=1) as tkb, \
         tc.tile_pool(name="w1stream", bufs=3) as w1s, \
         tc.tile_pool(name="w2stream", bufs=4) as w2s:
        ofT = tkb.tile([128, DT, TAIL], BF16, name="ofT")
        nc.sync.dma_start(ofT[:],
                          a2a_out[:].rearrange("(jt p) t -> p jt t", p=P))
        x2 = tkb.tile([128, 2, D], F32, name="x2")
        nc.sync.dma_start(x2[:],
                          x_res.ap().rearrange("(n p) d -> p n d", p=P))

        op_bank = tps.tile([128, 512], F32, name="op_bank")
        for tt2 in range(2):
            for nb in range(2):
                nsl = slice(nb * 512, (nb + 1) * 512)
                for jt in range(DT):
                    nc.tensor.matmul(op_bank[:],
                                     ofT[:, jt, tt2 * 128:(tt2 + 1) * 128],
                                     wo_sb[:, jt, nsl],
                                     start=(jt == 0), stop=(jt == DT - 1))
                nc.vector.tensor_tensor(x2[:, tt2, nsl], op_bank[:],
                                        x2[:, tt2, nsl], ALU.add)
        if (d := dump("x2", [128, 2 * D])) is not None:
            nc.sync.dma_start(d.ap().rearrange("p (n f) -> p n f", n=2),
                              x2[:])

        # LN2 (rsqrt via Ln/Exp) + PE-transpose h2 -> h2T
        h2T = tkb.tile([128, DT, TAIL], BF16, name="h2T")
        for tt2 in range(2):
            x2t = x2[:, tt2, :]
            ssum = tsb.tile([128, 1], F32, name="ssum2")
            nc.vector.tensor_reduce(ssum[:], x2t, AX.X, ALU.add)
            sq = tsb.tile([128, D], BF16, name="sq2")
            ssq = tsb.tile([128, 1], F32, name="ssq2")
            nc.scalar.activation(sq[:], x2t, AF.Square, accum_out=ssq[:])
            mu = tsb.tile([128, 1], F32, name="mu2")
            nc.vector.tensor_scalar_mul(mu[:], ssum[:], 1.0 / D)
            var = tsb.tile([128, 1], F32, name="var2")
            nc.vector.tensor_tensor(var[:], mu[:], mu[:], ALU.mult)
            ex2 = tsb.tile([128, 1], F32, name="ex22")
            nc.vector.tensor_scalar_mul(ex2[:], ssq[:], 1.0 / D)
            nc.vector.tensor_tensor(var[:], ex2[:], var[:], ALU.subtract)
            lnv2 = tsb.tile([128, 1], F32, name="lnv2")
            nc.scalar.activation(lnv2[:], var[:], AF.Ln, bias=eps_sb[:])
            r2 = tsb.tile([128, 1], F32, name="r2")
            nc.scalar.activation(r2[:], lnv2[:], AF.Exp, scale=-0.5)
            nrmu = tsb.tile([128, 1], F32, name="nrmu2")
            nc.vector.tensor_tensor(nrmu[:], r2[:], mu[:], ALU.mult)
            nc.vector.tensor_scalar_mul(nrmu[:], nrmu[:], -1.0)
            h2t = tsb.tile([128, D], BF16, name="h2t")
            nc.scalar.activation(h2t[:], x2t, AF.Identity,
                                 bias=nrmu[:], scale=r2[:])
            h2ps = tps.tile([128, DT, 128], BF16, name="h2ps")
            for dt in range(DT):
                nc.tensor.transpose(h2ps[:, dt, :],
                                    h2t[:, dt * 128:(dt + 1) * 128],
                                    ident_sb[:])
                nc.vector.tensor_copy(h2T[:, dt, tt2 * 128:(tt2 + 1) * 128],
                                      h2ps[:, dt, :])

        # MLP1: zT = gelu(W1^T h2 + b1) directly in [m, t] layout
        zT = tkb.tile([128, MT, TAIL], BF16, name="zT")
        for mc in range(8):
            mcsl = slice(mc * 512, (mc + 1) * 512)
            w1t = w1s.tile([128, DT, 512], BF16, name="w1t")
            nc.sync.dma_start(
                w1t[:], w1.ap().rearrange("(dt p) m -> p dt m", p=P)
                [:, :, mcsl])
            for ms in range(4):
                mt = mc * 4 + ms
                msl = slice(ms * 128, (ms + 1) * 128)
                zq = mps.tile([128, TAIL], F32, name="zq")
                for dt in range(DT):
                    nc.tensor.matmul(zq[:], w1t[:, dt, msl], h2T[:, dt, :],
                                     start=(dt == 0), stop=(dt == DT - 1))
                nc.scalar.activation(zT[:, mt, :], zq[:], AF.Gelu,
                                     bias=b1_sb[:, mt:mt + 1], scale=1.0)

        # MLP2: y2 = z @ w2, accumulate over mt into 4 resident banks
        y2_banks = [tps.tile([128, 512], F32, name=f"y2b{i}")
                    for i in range(4)]
        for mt in range(MT):
            w2t = w2s.tile([128, D], BF16, name="w2t")
            nc.sync.dma_start(
                w2t[:], w2.ap().rearrange("(n p) d -> n p d", p=P)[mt])
            for tt2 in range(2):
                for nb in range(2):
                    nc.tensor.matmul(
                        y2_banks[tt2 * 2 + nb],
                        zT[:, mt, tt2 * 128:(tt2 + 1) * 128],
                        w2t[:, nb * 512:(nb + 1) * 512],
                        start=(mt == 0), stop=(mt == MT - 1))
        for tt2 in range(2):
            for nb in range(2):
                nsl = slice(nb * 512, (nb + 1) * 512)
                ys = tsb.tile([128, 512], F32, name="ys")
                nc.vector.tensor_tensor(ys[:], y2_banks[tt2 * 2 + nb],
                                        x2[:, tt2, nsl], ALU.add)
                nc.sync.dma_start(
                    y_out.ap().rearrange("(n p) d -> p n d", p=P)
                    [:, tt2, nsl], ys[:])

    for pool in (dram, persist, const):
        pool.release()


def _build():
    nc = bacc.Bacc("TRN2", target_bir_lowering=False, debug=False,
                   num_devices=N_CORES)

    def din(name, shape, dt=BF16):
        return nc.dram_tensor(name, shape, dt, kind="ExternalInput")

    io = dict(
        x_bf=din("x_bf", [TOK, D]),
        x_res=din("x_res", [TAIL, D], F32),
        wq=din("wq", [D, 128]), wk=din("wk", [D, 128]),
        wv=din("wv", [D, 128]), wf=din("wf", [D, 128]),
        bqp=din("bqp", [128, 1], F32), bkp=din("bkp", [128, 1], F32),
        bvp=din("bvp", [128, 1], F32), bfp=din("bfp", [128, 1], F32),
        wo=din("wo", [D, D]),
        w1=din("w1", [D, MLP]),
        b1v=din("b1v", [128, MLP // 128], F32),
        w2=din("w2", [MLP, D]),
        ltriT=din("ltriT", [128, 128], F32),
        onescol=din("onescol", [128, 1], F32),
        cmask=din("cmask", [128, 128]),
        ident=din("ident", [128, 128]),
        bd128=din("bd128", [128, 128]),
        ones_row=din("ones_row", [1, 128]),
        y_out=nc.dram_tensor("y_out", [TAIL, D], F32, kind="ExternalOutput"),
    )

    dbg = [s for s in os.environ.get("GSA_DEBUG", "").split(",") if s]
    dbg_outs = {}

    def dump(name, shape, dt=F32):
        if name in dbg:
            t = nc.dram_tensor("dbg_" + name, shape, dt,
                               kind="ExternalOutput")
            dbg_outs[name] = t
            return t
        return None

    io["dump"] = dump
    with tile.TileContext(nc) as tcx:
        _emit(nc, tcx, io)
    nc.compile()
    return nc, sorted(dbg_outs)


def _host_prep(inputs):
    """Fold norms/biases into weights; build per-core in_maps."""
    f32 = np.float32
    bf16 = ml_dtypes.bfloat16
    x = np.asarray(inputs["hidden_states"], f32).reshape(TOK, D)
    ln1_w = np.asarray(inputs["ln1_w"], f32)
    ln1_b = np.asarray(inputs["ln1_b"], f32)
    ln2_w = np.asarray(inputs["ln2_w"], f32)
    ln2_b = np.asarray(inputs["ln2_b"], f32)
    gnorm = np.asarray(inputs["gnorm_w"], f32)
    Wq = np.asarray(inputs["Wq"], f32) * ln1_w[:, None]
    Wk = np.asarray(inputs["Wk"], f32) * ln1_w[:, None]
    Wv = np.asarray(inputs["Wv"], f32) * ln1_w[:, None]
    Wf = np.asarray(inputs["Wf"], f32) * ln1_w[:, None]
    bq = ln1_b @ np.asarray(inputs["Wq"], f32)
    bk = ln1_b @ np.asarray(inputs["Wk"], f32)
    bv = ln1_b @ np.asarray(inputs["Wv"], f32)
    bf_ = ln1_b @ np.asarray(inputs["Wf"], f32)
    Wo = np.asarray(inputs["Wo"], f32) * np.tile(gnorm, H)[:, None]
    W1 = np.asarray(inputs["W1"], f32) * ln2_w[:, None]
    b1 = np.asarray(inputs["b1"], f32) + ln2_b @ np.asarray(inputs["W1"], f32)
    W2 = np.asarray(inputs["W2"], f32)
    b2 = np.asarray(inputs["b2"], f32)

    tri = np.tril(np.ones((128, 128), f32))  # [t, tau] tau<=t
    ltriT = np.ascontiguousarray((-0.125 * tri).T)           # [tau, t]
    cmask = np.ascontiguousarray(tri.T.astype(bf16))         # [tau, t]
    ident = np.eye(128, dtype=bf16)
    bd128 = np.kron(np.eye(2, dtype=f32),
                    np.ones((64, 64), f32)).astype(bf16)
    common = dict(
        x_bf=np.ascontiguousarray(x.astype(bf16)),
        ltriT=ltriT,
        onescol=np.full((128, 1), -0.125, f32),
        cmask=cmask, ident=ident, bd128=bd128,
        ones_row=np.ones((1, 128), bf16),
        wo=np.ascontiguousarray(Wo.astype(bf16)),
        w1=np.ascontiguousarray(W1.astype(bf16)),
        b1v=np.ascontiguousarray(b1.reshape(MLP // 128, 128).T.astype(f32)),
        w2=np.ascontiguousarray(W2.astype(bf16)),
    )
    in_maps = []
    for r in range(N_CORES):
        jsl = slice(r * 128, (r + 1) * 128)  # 2 heads = 128 cols
        m = dict(common)
        m["x_res"] = np.ascontiguousarray(x[r * TAIL:(r + 1) * TAIL]
                                          + b2[None, :])
        m["wq"] = np.ascontiguousarray(Wq[:, jsl].astype(bf16))
        m["wk"] = np.ascontiguousarray(Wk[:, jsl].astype(bf16))
        m["wv"] = np.ascontiguousarray(Wv[:, jsl].astype(bf16))
        m["wf"] = np.ascontiguousarray(Wf[:, jsl].astype(bf16))
        m["bqp"] = np.ascontiguousarray(bq[jsl].reshape(128, 1))
        m["bkp"] = np.ascontiguousarray(bk[jsl].reshape(128, 1))
        m["bvp"] = np.ascontiguousarray(bv[jsl].reshape(128, 1))
        m["bfp"] = np.ascontiguousarray(bf_[jsl].reshape(128, 1))
        in_maps.append(m)
    return in_maps


def kernel(**inputs):
    if "nc" not in _cache:
        _cache["nc"], _cache["dbg"] = _build()
    nc = _cache["nc"]
    in_maps = _host_prep(inputs)
    res = run_bass_kernel_spmd(nc, in_maps, core_ids=list(range(N_CORES)),
                               trace=bool(os.environ.get("GSA_TRACE")))
    _cache["last_results"] = res
    out = np.concatenate([res.results[r]["y_out"] for r in range(N_CORES)],
                         axis=0)
    return out.reshape(B, T, D)


# revision 30
# speedup vs baseline: 1.4499x; 1.0756x over previous
"""GSA video block kernel for 8 TRN2 NeuronCores.

Sharding: head-parallel attention (2 heads/core) -> one AllToAll that
redistributes the RMS-normed head outputs from head-sharded to
token-sharded -> token-parallel tail (out-proj + LN2 + MLP with full
weights, 256 tokens/core).

The sequential T=512 gated-slot-attention scan is replaced by an exact
chunk-parallel formulation (C=128): intra-chunk terms via causal-masked
matmuls with per-slot decay factors, inter-chunk via carried states
K[DK,M] / V[M,DV].

v2: all layout changes (h, k/v/f, h2, z transposes) are done with PE
transposes + PSUM copies instead of DRAM-roundtrip DMA transposes;
MLP1 produces zT directly; rsqrt via Ln/Exp keeps one activation table
loaded through the scan; elementwise work is spread over
vector/scalar/gpsimd.
"""

import os
import sys

import numpy as np
import ml_dtypes

if "/opt/trn_rl_repo" not in sys.path:
    sys.path.insert(0, "/opt/trn_rl_repo")

import concourse.bass as bass  # noqa: E402
import concourse.mybir as mybir  # noqa: E402
import concourse.tile as tile  # noqa: E402
from concourse import bacc  # noqa: E402
from concourse.bass_utils import run_bass_kernel_spmd  # noqa: E402

BF16 = mybir.dt.bfloat16
F32 = mybir.dt.float32
AF = mybir.ActivationFunctionType
ALU = mybir.AluOpType
AX = mybir.AxisListType

B, T, D = 4, 512, 1024
H, DK, DV, M = 16, 64, 64, 64
MLP = 4096
EPS = 1e-6

N_CORES = 8
C = 128                    # scan chunk length
NCH = T // C               # chunks per batch = 4
TOK = B * T                # 2048 flat tokens
TT = TOK // 128            # 16 token tiles
DT = D // 128              # 8 d tiles
MT = MLP // 128            # 32 mlp tiles
TAIL = TOK // N_CORES      # 256 tokens per core in the tail
LN8 = float(np.log(0.125))
RG = [list(range(N_CORES))]

_cache = {}


def _emit(nc, tc, io):
    x_bf, x_res = io["x_bf"], io["x_res"]
    wq, wk, wv, wf = io["wq"], io["wk"], io["wv"], io["wf"]
    bqp, bkp = io["bqp"], io["bkp"]
    bvp, bfp = io["bvp"], io["bfp"]
    wo, w1, b1v, w2 = io["wo"], io["w1"], io["b1v"], io["w2"]
    ltriT, onescol, cmask = io["ltriT"], io["onescol"], io["cmask"]
    ident, bd128, ones_row = io["ident"], io["bd128"], io["ones_row"]
    y_out, dump = io["y_out"], io["dump"]
    P = 128

    const = tc.alloc_tile_pool(name="const", bufs=1)
    persist = tc.alloc_tile_pool(name="persist", bufs=1)
    dram = tc.alloc_tile_pool(name="dram", bufs=1, space="DRAM")

    # ---- warmup collective (prepay ncfw handshake) -----------------------
    wa_in = dram.tile([8, 128], BF16, name="wa_in")
    wa_out = dram.tile([8, 128], BF16, name="wa_out")
    nc.gpsimd.collective_compute("AllReduce", ALU.add, replica_groups=RG,
                                 ins=[wa_in.opt()], outs=[wa_out.opt()])

    # ---- constants into SBUF --------------------------------------------
    def cload(ap, shape, dt, name):
        t = const.tile(shape, dt, name=name)
        nc.sync.dma_start(t[:], ap)
        return t

    ltriT_sb = cload(ltriT.ap(), [128, 128], F32, "ltriT")
    onescol_sb = cload(onescol.ap(), [128, 1], F32, "onescol")
    cmask_sb = cload(cmask.ap(), [128, 128], BF16, "cmask")
    ident_sb = cload(ident.ap(), [128, 128], BF16, "ident")
    bd128_sb = cload(bd128.ap(), [128, 128], BF16, "bd128")
    ones_row_sb = cload(ones_row.ap(), [1, 128], BF16, "ones_row")
    bqp_sb = cload(bqp.ap(), [128, 1], F32, "bqp")
    bkp_sb = cload(bkp.ap(), [128, 1], F32, "bkp")
    bvp_sb = cload(bvp.ap(), [128, 1], F32, "bvp")
    bfp_sb = cload(bfp.ap(), [128, 1], F32, "bfp")
    b1_sb = cload(b1v.ap(), [128, MT], F32, "b1")
    eps_sb = const.tile([128, 1], F32)
    nc.vector.memset(eps_sb[:], EPS)
    ln8_sb = const.tile([128, 1], F32)
    nc.vector.memset(ln8_sb[:], LN8)

    wq_sb = const.tile([128, DT, 128], BF16)
    nc.sync.dma_start(wq_sb[:], wq.ap().rearrange("(dt p) j -> p dt j", p=P))
    wk_sb = const.tile([128, DT, 128], BF16)
    nc.sync.dma_start(wk_sb[:], wk.ap().rearrange("(dt p) j -> p dt j", p=P))
    wv_sb = const.tile([128, DT, 128], BF16)
    nc.sync.dma_start(wv_sb[:], wv.ap().rearrange("(dt p) j -> p dt j", p=P))
    wf_sb = const.tile([128, DT, 128], BF16)
    nc.sync.dma_start(wf_sb[:], wf.ap().rearrange("(dt p) j -> p dt j", p=P))
    wo_sb = const.tile([128, DT, D], BF16)
    nc.sync.dma_start(wo_sb[:], wo.ap().rearrange("(dt p) j -> p dt j", p=P))

    # ---- persistent activation tensors ----------------------------------
    qT = persist.tile([128, TOK], BF16, name="qT")       # [2h*64 dk, t]
    kT = persist.tile([128, TOK], BF16, name="kT")
    k_tm = persist.tile([128, TT, 128], BF16, name="k_tm")   # [t, 2h*64]
    v_tm = persist.tile([128, TT, 128], BF16, name="v_tm")
    sp = persist.tile([128, TT, 128], F32, name="sp")        # softplus(-f)
    s_tm = persist.tile([128, TT, 128], BF16, name="s_tm")   # 1-exp(g)
    onT = persist.tile([128, TOK], BF16, name="onT")         # normed oT

    a2a_in = [dram.tile([128 * N_CORES, 128], BF16, name=f"a2a_in{i}")
              for i in range(2)]
    a2a_out = [dram.tile([128 * N_CORES, 128], BF16, name=f"a2a_out{i}")
               for i in range(2)]

    p2h = tc.alloc_tile_pool(name="p2h", bufs=1)
    hT = p2h.tile([128, DT, TOK], BF16, name="hT")
    vT = p2h.tile([128, TOK], BF16, name="vT")
    fT = p2h.tile([128, TOK], BF16, name="fT")
    f_tm = p2h.tile([128, TT, 128], BF16, name="f_tm")

    # =====================================================================
    # P1: LN1 stats (batched), then per-tc: normalize + PE-transpose to hT
    #     + q/k projections (Silu era)
    # =====================================================================
    p0 = tc.alloc_tile_pool(name="p0", bufs=1)
    x_sb = p0.tile([128, TT, D], BF16, name="x_sb")
    stats = p0.tile([128, 2, TT], F32, name="stats")  # [.,0,:]=r [.,1,:]=nrmu
    for tt in range(TT):
        nc.sync.dma_start(
            x_sb[:, tt, :],
            x_bf.ap().rearrange("(n p) d -> n p d", p=P)[tt])
    with tc.tile_pool(name="p1", bufs=3) as p1, \
         tc.tile_pool(name="p1s", bufs=2) as p1s, \
         tc.tile_pool(name="htp", bufs=3, space="PSUM") as htp, \
         tc.tile_pool(name="proj_ps", bufs=2, space="PSUM") as pps:
        musb = p1.tile([128, TT], F32, name="musb")
        sssb = p1.tile([128, TT], F32, name="sssb")
        for tt in range(TT):
            nc.vector.tensor_reduce(musb[:, tt:tt + 1], x_sb[:, tt, :],
                                    AX.X, ALU.add)
            sq = p1s.tile([128, D], BF16, name="sq")
            nc.scalar.activation(sq[:], x_sb[:, tt, :], AF.Square,
                                 accum_out=sssb[:, tt:tt + 1])
        mu = p1.tile([128, TT], F32, name="mu")
        nc.vector.tensor_scalar_mul(mu[:], musb[:], 1.0 / D)
        var = p1.tile([128, TT], F32, name="var")
        nc.vector.tensor_tensor(var[:], mu[:], mu[:], ALU.mult)
        ex2 = p1.tile([128, TT], F32, name="ex2")
        nc.vector.tensor_scalar_mul(ex2[:], sssb[:], 1.0 / D)
        nc.vector.tensor_tensor(var[:], ex2[:], var[:], ALU.subtract)
        lnv = p1.tile([128, TT], F32, name="lnv")
        nc.scalar.activation(lnv[:], var[:], AF.Ln, bias=eps_sb[:])
        nc.scalar.activation(stats[:, 0, :], lnv[:], AF.Exp, scale=-0.5)
        nc.vector.tensor_tensor(stats[:, 1, :], stats[:, 0, :], mu[:],
                                ALU.mult)
        nc.vector.tensor_scalar_mul(stats[:, 1, :], stats[:, 1, :], -1.0)

        def proj(dst, w_sb, bias, fn, tcsl):
            bank = pps.tile([128, 512], F32, name="projbank")
            for dt in range(DT):
                nc.tensor.matmul(bank[:], w_sb[:, dt, :], hT[:, dt, tcsl],
                                 start=(dt == 0), stop=(dt == DT - 1))
            nc.scalar.activation(dst[:, tcsl], bank[:], fn,
                                 bias=bias[:], scale=1.0)

        for tci in range(4):
            for tt in range(tci * 4, tci * 4 + 4):
                ht = p1.tile([128, D], BF16, name="ht")
                nc.scalar.activation(ht[:], x_sb[:, tt, :], AF.Identity,
                                     bias=stats[:, 1, tt:tt + 1],
                                     scale=stats[:, 0, tt:tt + 1])
                ttsl = slice(tt * 128, (tt + 1) * 128)
                for dh in range(2):
                    tp = htp.tile([128, 4, 128], BF16, name="htps")
                    for dj in range(4):
                        dt = dh * 4 + dj
                        nc.tensor.transpose(
                            tp[:, dj, :], ht[:, dt * 128:(dt + 1) * 128],
                            ident_sb[:])
                        nc.vector.tensor_copy(hT[:, dt, ttsl], tp[:, dj, :])
            tcsl = slice(tci * 512, (tci + 1) * 512)
            proj(qT, wq_sb, bqp_sb, AF.Silu, tcsl)
            proj(kT, wk_sb, bkp_sb, AF.Silu, tcsl)

        # ================================================================
        # P2: v/f projections + PE transposes to token-major + gates
        # ================================================================
        with tc.tile_pool(name="kvftp", bufs=3, space="PSUM") as kvftp, \
             tc.tile_pool(name="sgate", bufs=2) as sg:
            for tci in range(4):
                tcsl = slice(tci * 512, (tci + 1) * 512)
                proj(vT, wv_sb, bvp_sb, AF.Identity, tcsl)
                proj(fT, wf_sb, bfp_sb, AF.Identity, tcsl)
                for tt in range(tci * 4, tci * 4 + 4):
                    ttsl = slice(tt * 128, (tt + 1) * 128)
                    tp = kvftp.tile([128, 3, 128], BF16, name="kvfps")
                    nc.tensor.transpose(tp[:, 0, :], kT[:, ttsl], ident_sb[:])
                    nc.vector.tensor_copy(k_tm[:, tt, :], tp[:, 0, :])
                    nc.tensor.transpose(tp[:, 1, :], vT[:, ttsl], ident_sb[:])
                    nc.scalar.activation(v_tm[:, tt, :], tp[:, 1, :],
                                         AF.Identity)
                    nc.tensor.transpose(tp[:, 2, :], fT[:, ttsl], ident_sb[:])
                    nc.vector.tensor_copy(f_tm[:, tt, :], tp[:, 2, :])
            enf = sg.tile([128, TT, 128], F32, name="enf")
            nc.scalar.activation(enf[:], f_tm[:], AF.Exp, scale=-1.0)
            nc.scalar.activation(sp[:], enf[:], AF.Ln, bias=1.0)
            e8 = sg.tile([128, TT, 128], BF16, name="e8")
            nc.scalar.activation(e8[:], sp[:], AF.Exp, scale=-0.125)
            nc.vector.tensor_scalar(s_tm[:], e8[:], -1.0, 1.0,
                                    ALU.mult, ALU.add)

    p0.release()
    p2h.release()

    for nm, t_sb in (("qT", qT), ("kT", kT)):
        if (d := dump(nm, [128, TOK], BF16)) is not None:
            nc.sync.dma_start(d.ap(), t_sb[:])
    for nm, t_sb in (("k_tm", k_tm), ("v_tm", v_tm), ("s_tm", s_tm)):
        if (d := dump(nm, [128, TT * 128], BF16)) is not None:
            nc.sync.dma_start(d.ap().rearrange("p (n f) -> p n f", n=TT),
                              t_sb[:])
    if (d := dump("sp", [128, TT * 128])) is not None:
        nc.sync.dma_start(d.ap().rearrange("p (n f) -> p n f", n=TT), sp[:])

    w1k = tc.alloc_tile_pool(name="w1k", bufs=1)
    w1_sb = w1k.tile([128, DT, MLP], BF16, name="w1_sb")
    nc.sync.dma_start(w1_sb[:], w1.ap().rearrange("(dt p) m -> p dt m", p=P))

    # =====================================================================
    # P3: chunked scan, b-major; RMS + write o_nT
    # =====================================================================
    with tc.tile_pool(name="scan_ps", bufs=2, space="PSUM") as sps, \
         tc.tile_pool(name="scan_sb", bufs=2) as ssb, \
         tc.tile_pool(name="state_sb", bufs=1) as stb:
        Kst4 = [stb.tile([128, 64], BF16, name=f"Kst{b}")
                for b in range(B)]   # [2h*64 dk, s]
        Vst4 = [stb.tile([128, 64], BF16, name=f"Vst{b}")
                for b in range(B)]   # [2h*64 s, dv]
        for c in range(NCH):
            for b in range(B):
                Kst, Vst = Kst4[b], Vst4[b]
                bi = b * 4 + c
                tsl = slice(b * 512 + c * 128, b * 512 + (c + 1) * 128)
                first = (c == 0)
                bankA = sps.tile([128, 512], F32, name="bankA")
                ps_b = bankA[:, 0:128]
                ps_lc = bankA[:, 256:257]
                ps_lcr = bankA[0:1, 257:385]
                ps_lambc = bankA[:, 384:512]
                bankB = sps.tile([128, 512], F32, name="bankB")
                ps_a = (bankB[:, 0:128], bankB[:, 128:256])
                ps_ok = bankB[:, 256:384]
                bankD = sps.tile([128, 1024], BF16, name="bankD")
                ps_pt = bankD[:, 0:128]
                ps_st = bankD[:, 128:256]
                ps_b2 = ps_a  # reused after am is extracted
                bankF = sps.tile([128, 512], F32, name="bankF")
                ps_o = (bankF[0:64, 0:128], bankF[0:64, 128:256])
                ps_dk = (bankF[0:64, 256:320], bankF[0:64, 320:384])
                ps_dv = (bankF[0:64, 384:448], bankF[0:64, 448:512])

                # cumsum b = ltriT.T @ sp (f32); colsum -> bCT
                nc.tensor.matmul(ps_b, ltriT_sb[:], sp[:, bi, :],
                                 start=True, stop=True)
                nc.tensor.matmul(ps_lc, sp[:, bi, :], onescol_sb[:],
                                 start=True, stop=True)
                lam = ssb.tile([128, 128], BF16, name="lam")
                nc.scalar.activation(lam[:], ps_b, AF.Exp)
                lam_s = ssb.tile([128, 128], BF16, name="lam_s")
                nc.scalar.activation(lam_s[:], ps_b, AF.Exp, bias=ln8_sb[:])
                en = ssb.tile([128, 128], BF16, name="en")
                nc.scalar.activation(en[:], ps_b, AF.Exp, scale=-1.0)
                lamCT = ssb.tile([128, 1], F32, name="lamCT")
                nc.scalar.activation(lamCT[:], ps_lc, AF.Exp)
                nc.tensor.matmul(ps_lcr, onescol_sb[:], sp[:, bi, :],
                                 start=True, stop=True)
                lamCr = ssb.tile([1, 128], BF16, name="lamCr")
                nc.scalar.activation(lamCr[:], ps_lcr, AF.Exp)

                s_til = ssb.tile([128, 128], BF16, name="s_til")
                nc.vector.tensor_tensor(s_til[:], s_tm[:, bi, :], en[:],
                                        ALU.mult)
                nc.tensor.matmul(ps_lambc, ones_row_sb[:], lamCr[:],
                                 start=True, stop=True)
                s2 = ssb.tile([128, 128], BF16, name="s2")
                nc.vector.tensor_tensor(s2[:], s_til[:], ps_lambc, ALU.mult)

                am = ssb.tile([128, 256], BF16, name="am")
                for h in range(2):
                    hs = slice(h * 64, (h + 1) * 64)
                    nc.tensor.matmul(ps_a[h], kT[hs, tsl], qT[hs, tsl],
                                     start=True, stop=True)
                    nc.vector.tensor_tensor(am[:, h * 128:(h + 1) * 128],
                                            ps_a[h], cmask_sb[:], ALU.mult)
                for h in range(2):
                    hs = slice(h * 64, (h + 1) * 64)
                    oks = ps_ok[:, h * 64:(h + 1) * 64]
                    if not first:
                        nc.tensor.matmul(oks, qT[hs, tsl], Kst[hs, :],
                                         start=True, stop=False)
                    nc.tensor.matmul(oks, am[:, h * 128:(h + 1) * 128],
                                     s_til[:, h * 64:(h + 1) * 64],
                                     start=first, stop=True)
                # softmax over slots (per head), pl = P * lam
                oksc = ssb.tile([128, 128], F32, name="oksc")
                nc.vector.tensor_tensor(oksc[:], ps_ok, lam_s[:], ALU.mult)
                ex = ssb.tile([128, 128], BF16, name="ex")
                nc.scalar.activation(ex[:], oksc[:], AF.Exp)
                rsum = ssb.tile([128, 2], F32, name="rsum")
                nc.vector.tensor_reduce(
                    rsum[:], ex[:].rearrange("p (h s) -> p h s", h=2),
                    AX.X, ALU.add)
                rcp = ssb.tile([128, 2], F32, name="rcp")
                nc.vector.reciprocal(rcp[:], rsum[:])
                pl = ssb.tile([128, 128], BF16, name="pl")
                nc.vector.tensor_tensor(pl[:], ex[:], lam[:], ALU.mult)
                nc.vector.tensor_tensor(
                    pl[:].rearrange("p (h s) -> p h s", h=2),
                    pl[:].rearrange("p (h s) -> p h s", h=2),
                    rcp[:].rearrange("p (h o) -> p h o", h=2)
                        .to_broadcast([128, 2, 64]),
                    ALU.mult)

                # transposes: plT, s_tilT  [2h*64 s, 128 t]
                plT = ssb.tile([128, 128], BF16, name="plT")
                s_tilT = ssb.tile([128, 128], BF16, name="s_tilT")
                nc.tensor.transpose(ps_pt, pl[:], ident_sb[:])
                nc.scalar.activation(plT[:], ps_pt, AF.Identity)
                nc.tensor.transpose(ps_st, s_til[:], ident_sb[:])
                nc.vector.tensor_copy(s_tilT[:], ps_st)

                b2m = ssb.tile([128, 256], BF16, name="b2m")
                for h in range(2):
                    hs = slice(h * 64, (h + 1) * 64)
                    nc.tensor.matmul(ps_b2[h], s_tilT[hs, :], plT[hs, :],
                                     start=True, stop=True)
                    nc.vector.tensor_tensor(b2m[:, h * 128:(h + 1) * 128],
                                            ps_b2[h], cmask_sb[:], ALU.mult)
                for h in range(2):
                    hs = slice(h * 64, (h + 1) * 64)
                    if not first:
                        nc.tensor.matmul(ps_o[h], Vst[hs, :], plT[hs, :],
                                         start=True, stop=False)
                    nc.tensor.matmul(ps_o[h], v_tm[:, bi, hs],
                                     b2m[:, h * 128:(h + 1) * 128],
                                     start=first, stop=True)
                    nc.tensor.matmul(ps_dk[h], k_tm[:, bi, hs],
                                     s2[:, hs], start=True, stop=True)
                    nc.tensor.matmul(ps_dv[h], s2[:, hs], v_tm[:, bi, hs],
                                     start=True, stop=True)
                    if first:
                        nc.vector.tensor_copy(Kst[hs, :], ps_dk[h])
                        nc.scalar.activation(Vst[hs, :], ps_dv[h],
                                             AF.Identity)
                    else:
                        nc.vector.tensor_tensor(
                            Kst[hs, :], Kst[hs, :],
                            ps_lambc[hs, hs], ALU.mult)
                        nc.vector.tensor_tensor(Kst[hs, :], Kst[hs, :],
                                                ps_dk[h], ALU.add)
                        nc.vector.tensor_scalar(Vst[hs, :], Vst[hs, :],
                                                lamCT[hs, 0:1], None,
                                                ALU.mult)
                        nc.vector.tensor_tensor(Vst[hs, :], Vst[hs, :],
                                                ps_dv[h], ALU.add)

                # write raw oT (RMS batched after the loop)
                nc.scalar.activation(onT[0:64, tsl], ps_o[0], AF.Identity)
                nc.scalar.activation(onT[64:128, tsl], ps_o[1], AF.Identity)

            # after steps 1 and 3: RMS-norm the finished chunk pair, then
            # AllToAll that half to its tail owners
            if c in (1, 3):
                half = c // 2
                sqo = ssb.tile([128, 8, 128], BF16, name="sqo")
                for b2_ in range(B):
                    for k in range(2):
                        ci = 2 * half + k
                        tslb = slice(b2_ * 512 + ci * 128,
                                     b2_ * 512 + (ci + 1) * 128)
                        nc.vector.tensor_tensor(sqo[:, b2_ * 2 + k, :],
                                                onT[:, tslb], onT[:, tslb],
                                                ALU.mult)
                for i in range(2):
                    rb = sps.tile([128, 512], F32, name="bankF")
                    nc.tensor.matmul(rb[:], bd128_sb[:],
                                     sqo[:, i * 4:(i + 1) * 4, :],
                                     start=True, stop=True)
                    lno = ssb.tile([128, 512], F32, name="lno")
                    nc.scalar.activation(lno[:], rb[:], AF.Ln,
                                         bias=eps_sb[:], scale=1.0 / DV)
                    rro = ssb.tile([128, 512], F32, name="rro")
                    nc.scalar.activation(rro[:], lno[:], AF.Exp, scale=-0.5)
                    for sl in range(4):
                        b2_, k = divmod(i * 4 + sl, 2)
                        ci = 2 * half + k
                        tslb = slice(b2_ * 512 + ci * 128,
                                     b2_ * 512 + (ci + 1) * 128)
                        nc.vector.tensor_tensor(
                            onT[:, tslb], onT[:, tslb],
                            rro[:, sl * 128:(sl + 1) * 128], ALU.mult)
                for j in range(8):
                    bj, kj = divmod(j, 2)
                    ci = 2 * half + kj
                    tslb = slice(bj * 512 + ci * 128,
                                 bj * 512 + (ci + 1) * 128)
                    nc.sync.dma_start(
                        a2a_in[half][j * 128:(j + 1) * 128, :],
                        onT[:, tslb])
                nc.gpsimd.collective_compute(
                    "AllToAll", ALU.bypass, replica_groups=RG,
                    ins=[a2a_in[half].opt()], outs=[a2a_out[half].opt()])

    if (d := dump("onT", [128, TOK], BF16)) is not None:
        nc.sync.dma_start(d.ap(), onT[:])

    # =====================================================================
    # P4 tail: out-proj + residual + LN2 + MLP on 256 tokens
    # =====================================================================
    with tc.tile_pool(name="tail_ps", bufs=1, space="PSUM") as tps, \
         tc.tile_pool(name="mlp_ps", bufs=2, space="PSUM") as mps, \
         tc.tile_pool(name="tail_sb", bufs=2) as tsb, \
         tc.tile_pool(name="tail_keep", bufs=1) as tkb, \
         tc.tile_pool(name="w1stream", bufs=3) as w1s, \
         tc.tile_pool(name="w2stream", bufs=3) as w2s:
        ofT = tkb.tile([128, DT, TAIL], BF16, name="ofT")
        nc.sync.dma_start(ofT[:],
                          a2a_out[:].rearrange("(jt p) t -> p jt t", p=P))
        x2 = tkb.tile([128, 2, D], F32, name="x2")
        nc.sync.dma_start(x2[:],
                          x_res.ap().rearrange("(n p) d -> p n d", p=P))

        op_bank = tps.tile([128, 512], F32, name="op_bank")
        for tt2 in range(2):
            for nb in range(2):
                nsl = slice(nb * 512, (nb + 1) * 512)
                for jt in range(DT):
                    nc.tensor.matmul(op_bank[:],
                                     ofT[:, jt, tt2 * 128:(tt2 + 1) * 128],
                                     wo_sb[:, jt, nsl],
                                     start=(jt == 0), stop=(jt == DT - 1))
                nc.vector.tensor_tensor(x2[:, tt2, nsl], op_bank[:],
                                        x2[:, tt2, nsl], ALU.add)
        if (d := dump("x2", [128, 2 * D])) is not None:
            nc.sync.dma_start(d.ap().rearrange("p (n f) -> p n f", n=2),
                              x2[:])

        # LN2 (rsqrt via Ln/Exp) + PE-transpose h2 -> h2T
        h2T = tkb.tile([128, DT, TAIL], BF16, name="h2T")
        for tt2 in range(2):
            x2t = x2[:, tt2, :]
            ssum = tsb.tile([128, 1], F32, name="ssum2")
            nc.vector.tensor_reduce(ssum[:], x2t, AX.X, ALU.add)
            sq = tsb.tile([128, D], BF16, name="sq2")
            ssq = tsb.tile([128, 1], F32, name="ssq2")
            nc.scalar.activation(sq[:], x2t, AF.Square, accum_out=ssq[:])
            mu = tsb.tile([128, 1], F32, name="mu2")
            nc.vector.tensor_scalar_mul(mu[:], ssum[:], 1.0 / D)
            var = tsb.tile([128, 1], F32, name="var2")
            nc.vector.tensor_tensor(var[:], mu[:], mu[:], ALU.mult)
            ex2 = tsb.tile([128, 1], F32, name="ex22")
            nc.vector.tensor_scalar_mul(ex2[:], ssq[:], 1.0 / D)
            nc.vector.tensor_tensor(var[:], ex2[:], var[:], ALU.subtract)
            lnv2 = tsb.tile([128, 1], F32, name="lnv2")
            nc.scalar.activation(lnv2[:], var[:], AF.Ln, bias=eps_sb[:])
            r2 = tsb.tile([128, 1], F32, name="r2")
            nc.scalar.activation(r2[:], lnv2[:], AF.Exp, scale=-0.5)
            nrmu = tsb.tile([128, 1], F32, name="nrmu2")
            nc.vector.tensor_tensor(nrmu[:], r2[:], mu[:], ALU.mult)
            nc.vector.tensor_scalar_mul(nrmu[:], nrmu[:], -1.0)
            h2t = tsb.tile([128, D], BF16, name="h2t")
            nc.scalar.activation(h2t[:], x2t, AF.Identity,
                                 bias=nrmu[:], scale=r2[:])
            h2ps = tps.tile([128, DT, 128], BF16, name="h2ps")
            for dt in range(DT):
                nc.tensor.transpose(h2ps[:, dt, :],
                                    h2t[:, dt * 128:(dt + 1) * 128],
                                    ident_sb[:])
                nc.vector.tensor_copy(h2T[:, dt, tt2 * 128:(tt2 + 1) * 128],
                                      h2ps[:, dt, :])

        # MLP1: zT = gelu(W1^T h2 + b1) directly in [m, t] layout
        zT = tkb.tile([128, MT, TAIL], BF16, name="zT")
        for mc in range(8):
            mcsl = slice(mc * 512, (mc + 1) * 512)
            w1t = w1s.tile([128, DT, 512], BF16, name="w1t")
            nc.sync.dma_start(
                w1t[:], w1.ap().rearrange("(dt p) m -> p dt m", p=P)
                [:, :, mcsl])
            for ms in range(4):
                mt = mc * 4 + ms
                msl = slice(ms * 128, (ms + 1) * 128)
                zq = mps.tile([128, TAIL], F32, name="zq")
                for dt in range(DT):
                    nc.tensor.matmul(zq[:], w1t[:, dt, msl], h2T[:, dt, :],
                                     start=(dt == 0), stop=(dt == DT - 1))
                nc.scalar.activation(zT[:, mt, :], zq[:], AF.Gelu,
                                     bias=b1_sb[:, mt:mt + 1], scale=1.0)

        # MLP2: y2 = z @ w2, accumulate over mt into 4 resident banks
        y2_banks = [tps.tile([128, 512], F32, name=f"y2b{i}")
                    for i in range(4)]
        for mt in range(MT):
            w2t = w2s.tile([128, D], BF16, name="w2t")
            nc.sync.dma_start(
                w2t[:], w2.ap().rearrange("(n p) d -> n p d", p=P)[mt])
            for tt2 in range(2):
                for nb in range(2):
                    nc.tensor.matmul(
                        y2_banks[tt2 * 2 + nb],
                        zT[:, mt, tt2 * 128:(tt2 + 1) * 128],
                        w2t[:, nb * 512:(nb + 1) * 512],
                        start=(mt == 0), stop=(mt == MT - 1))
        for tt2 in range(2):
            for nb in range(2):
                nsl = slice(nb * 512, (nb + 1) * 512)
                ys = tsb.tile([128, 512], F32, name="ys")
                nc.vector.tensor_tensor(ys[:], y2_banks[tt2 * 2 + nb],
                                        x2[:, tt2, nsl], ALU.add)
                nc.sync.dma_start(
                    y_out.ap().rearrange("(n p) d -> p n d", p=P)
                    [:, tt2, nsl], ys[:])

    for pool in (dram, persist, const):
        pool.release()


def _build():
    nc = bacc.Bacc("TRN2", target_bir_lowering=False, debug=False,
                   num_devices=N_CORES)

    def din(name, shape, dt=BF16):
        return nc.dram_tensor(name, shape, dt, kind="ExternalInput")

    io = dict(
        x_bf=din("x_bf", [TOK, D]),
        x_res=din("x_res", [TAIL, D], F32),
        wq=din("wq", [D, 128]), wk=din("wk", [D, 128]),
        wv=din("wv", [D, 128]), wf=din("wf", [D, 128]),
        bqp=din("bqp", [128, 1], F32), bkp=din("bkp", [128, 1], F32),
        bvp=din("bvp", [128, 1], F32), bfp=din("bfp", [128, 1], F32),
        wo=din("wo", [D, D]),
        w1=din("w1", [D, MLP]),
        b1v=din("b1v", [128, MLP // 128], F32),
        w2=din("w2", [MLP, D]),
        ltriT=din("ltriT", [128, 128], F32),
        onescol=din("onescol", [128, 1], F32),
        cmask=din("cmask", [128, 128]),
        ident=din("ident", [128, 128]),
        bd128=din("bd128", [128, 128]),
        ones_row=din("ones_row", [1, 128]),
        y_out=nc.dram_tensor("y_out", [TAIL, D], F32, kind="ExternalOutput"),
    )

    dbg = [s for s in os.environ.get("GSA_DEBUG", "").split(",") if s]
    dbg_outs = {}

    def dump(name, shape, dt=F32):
        if name in dbg:
            t = nc.dram_tensor("dbg_" + name, shape, dt,
                               kind="ExternalOutput")
            dbg_outs[name] = t
            return t
        return None

    io["dump"] = dump
    with tile.TileContext(nc) as tcx:
        _emit(nc, tcx, io)
    nc.compile()
    return nc, sorted(dbg_outs)


def _host_prep(inputs):
    """Fold norms/biases into weights; build per-core in_maps."""
    f32 = np.float32
    bf16 = ml_dtypes.bfloat16
    x = np.asarray(inputs["hidden_states"], f32).reshape(TOK, D)
    ln1_w = np.asarray(inputs["ln1_w"], f32)
    ln1_b = np.asarray(inputs["ln1_b"], f32)
    ln2_w = np.asarray(inputs["ln2_w"], f32)
    ln2_b = np.asarray(inputs["ln2_b"], f32)
    gnorm = np.asarray(inputs["gnorm_w"], f32)
    Wq = np.asarray(inputs["Wq"], f32) * ln1_w[:, None]
    Wk = np.asarray(inputs["Wk"], f32) * ln1_w[:, None]
    Wv = np.asarray(inputs["Wv"], f32) * ln1_w[:, None]
    Wf = np.asarray(inputs["Wf"], f32) * ln1_w[:, None]
    bq = ln1_b @ np.asarray(inputs["Wq"], f32)
    bk = ln1_b @ np.asarray(inputs["Wk"], f32)
    bv = ln1_b @ np.asarray(inputs["Wv"], f32)
    bf_ = ln1_b @ np.asarray(inputs["Wf"], f32)
    Wo = np.asarray(inputs["Wo"], f32) * np.tile(gnorm, H)[:, None]
    W1 = np.asarray(inputs["W1"], f32) * ln2_w[:, None]
    b1 = np.asarray(inputs["b1"], f32) + ln2_b @ np.asarray(inputs["W1"], f32)
    W2 = np.asarray(inputs["W2"], f32)
    b2 = np.asarray(inputs["b2"], f32)

    tri = np.tril(np.ones((128, 128), f32))  # [t, tau] tau<=t
    ltriT = np.ascontiguousarray((-0.125 * tri).T)           # [tau, t]
    cmask = np.ascontiguousarray(tri.T.astype(bf16))         # [tau, t]
    ident = np.eye(128, dtype=bf16)
    bd128 = np.kron(np.eye(2, dtype=f32),
                    np.ones((64, 64), f32)).astype(bf16)
    common = dict(
        x_bf=np.ascontiguousarray(x.astype(bf16)),
        ltriT=ltriT,
        onescol=np.full((128, 1), -0.125, f32),
        cmask=cmask, ident=ident, bd128=bd128,
        ones_row=np.ones((1, 128), bf16),
        wo=np.ascontiguousarray(Wo.astype(bf16)),
        w1=np.ascontiguousarray(W1.astype(bf16)),
        b1v=np.ascontiguousarray(b1.reshape(MLP // 128, 128).T.astype(f32)),
        w2=np.ascontiguousarray(W2.astype(bf16)),
    )
    in_maps = []
    for r in range(N_CORES):
        jsl = slice(r * 128, (r + 1) * 128)  # 2 heads = 128 cols
        m = dict(common)
        idx = _tail_token_idx(r)
        m["x_res"] = np.ascontiguousarray(x[idx] + b2[None, :])
        m["wq"] = np.ascontiguousarray(Wq[:, jsl].astype(bf16))
        m["wk"] = np.ascontiguousarray(Wk[:, jsl].astype(bf16))
        m["wv"] = np.ascontiguousarray(Wv[:, jsl].astype(bf16))
        m["wf"] = np.ascontiguousarray(Wf[:, jsl].astype(bf16))
        m["bqp"] = np.ascontiguousarray(bq[jsl].reshape(128, 1))
        m["bkp"] = np.ascontiguousarray(bk[jsl].reshape(128, 1))
        m["bvp"] = np.ascontiguousarray(bv[jsl].reshape(128, 1))
        m["bfp"] = np.ascontiguousarray(bf_[jsl].reshape(128, 1))
        in_maps.append(m)
    return in_maps


def _tail_token_idx(r):
    """Global token indices owned by core r: chunks (b=r//2, c=r%2) and
    (b=r//2, c=r%2+2), 128 tokens each."""
    base = (r // 2) * 512 + (r % 2) * 128
    return np.concatenate([base + np.arange(128),
                           base + 256 + np.arange(128)])


def kernel(**inputs):
    if "nc" not in _cache:
        _cache["nc"], _cache["dbg"] = _build()
    nc = _cache["nc"]
    in_maps = _host_prep(inputs)
    res = run_bass_kernel_spmd(nc, in_maps, core_ids=list(range(N_CORES)),
                               trace=bool(os.environ.get("GSA_TRACE")))
    _cache["last_results"] = res
    out = np.empty((TOK, D), np.float32)
    for r in range(N_CORES):
        out[_tail_token_idx(r)] = res.results[r]["y_out"]
    return out.reshape(B, T, D)
